# revision 13
# baseline (speedup 1.0000x reference)
"""GNN encoder (ECCConv -> GATConv -> GlobalAvgPool -> Dense) on 8 trn2 NeuronCores.

Sharding: edge-parallel by *destination* node. Core c owns nodes
[c*6250, (c+1)*6250) and every edge whose dst lands in that range, so all
segment reductions (ECC aggregate, GAT softmax denom, GAT weighted sum) are
core-local. Edges are host-sorted by dst and grouped into 128-node windows;
segment-sums are PE matmuls against host-built one-hot tiles accumulated in
PSUM per window. Cross-core traffic: one AllGather of the per-node GAT table
[xp | a_neigh] and one tiny AllReduce of the pooled vector.
"""
import sys

for _p in ("/opt/trn_rl_repo", "/root/.axon_site/_ro/trn_rl_repo"):
    if _p not in sys.path:
        sys.path.append(_p)

import numpy as np

import concourse.bass as bass
import concourse.bacc as bacc
import concourse.tile as tile
import concourse.mybir as mybir
import concourse.bass_utils as bass_utils
from concourse.masks import make_identity

F32 = mybir.dt.float32
I32 = mybir.dt.int32

# problem constants (hardcoded per spec)
N = 50000
E = 100000
F_IN = 32
F_E = 8
F1 = 64
F2 = 64
KH = 32
FC = 32
NCORES = 8
NPC = N // NCORES          # 6250 nodes per core
WIN = 128                  # node window (one-hot matmul output partitions)
NWIN = (NPC + WIN - 1) // WIN   # 49
NPC_PAD = NWIN * WIN       # 6272
ST = 4                     # edge tiles per super-tile (op-overhead amortization)

_CACHE = {}


def _host_shard(edge_index):
    """Sort edges by dst, shard by dst//NPC, pad each (core,window) edge list
    to a tile count equalized across cores (SPMD needs identical structure)."""
    src = np.asarray(edge_index[0], np.int64)
    dst = np.asarray(edge_index[1], np.int64)
    core = dst // NPC
    dst_local = dst - core * NPC
    win = dst_local // WIN

    per_core_edges = []       # list of edge-id arrays, sorted by dst
    cnt = np.zeros((NCORES, NWIN), np.int64)
    for c in range(NCORES):
        ids = np.nonzero(core == c)[0]
        ids = ids[np.argsort(dst_local[ids], kind="stable")]
        per_core_edges.append(ids)
        cnt[c] = np.bincount(win[ids], minlength=NWIN)

    tiles_per_win = np.maximum(np.ceil(cnt / 128).astype(np.int64).max(axis=0), 0)
    # round total tiles up to a multiple of ST so super-tiles are uniform:
    # pad by growing the last window's tile count
    ntiles = int(tiles_per_win.sum())
    if ntiles % ST:
        tiles_per_win[-1] += ST - ntiles % ST
        ntiles = int(tiles_per_win.sum())
    e_pad = ntiles * 128

    win_of_tile = np.repeat(np.arange(NWIN), tiles_per_win)
    slot_base = np.concatenate([[0], np.cumsum(tiles_per_win * 128)])

    # per-core padded slot arrays
    eid = np.full((NCORES, e_pad), -1, np.int64)       # -1 = padding slot
    for c in range(NCORES):
        ids = per_core_edges[c]
        w_ids = win[ids]
        for w in range(NWIN):
            wi = ids[w_ids == w]
            eid[c, slot_base[w]: slot_base[w] + len(wi)] = wi
    return eid, win_of_tile, tiles_per_win, ntiles, e_pad, src, dst, dst_local


def _host_inputs(inputs):
    x = np.ascontiguousarray(np.asarray(inputs["x"], np.float32))
    e = np.ascontiguousarray(np.asarray(inputs["e"], np.float32))
    eid, win_of_tile, tiles_per_win, ntiles, e_pad, src, dst, dst_local = _host_shard(
        inputs["edge_index"])

    w0 = np.asarray(inputs["ecc_w0"], np.float32)
    b0 = np.asarray(inputs["ecc_b0"], np.float32)
    w1 = np.asarray(inputs["ecc_w1"], np.float32)
    b1 = np.asarray(inputs["ecc_b1"], np.float32)
    root = np.asarray(inputs["ecc_root"], np.float32)
    ecc_bias = np.asarray(inputs["ecc_bias"], np.float32)
    gk = np.asarray(inputs["gat_kernel"], np.float32)
    a_s = np.asarray(inputs["gat_attn_self"], np.float32)
    a_n = np.asarray(inputs["gat_attn_neigh"], np.float32)
    gat_bias = np.asarray(inputs["gat_bias"], np.float32)
    fc_w = np.asarray(inputs["fc_w"], np.float32)
    fc_b = np.asarray(inputs["fc_b"], np.float32)

    use_b0 = bool(np.any(b0))
    use_b1 = bool(np.any(b1))
    ke = F_E + 1 if use_b0 else F_E
    nchunk = 9 if use_b1 else 8

    # --- shared (identical on every core) parameter tensors ---
    w0m = np.vstack([w0, b0[None, :]]) if use_b0 else w0            # [ke, KH]
    # W1r[(k*F_IN+i), o] = w1[k, i*F1+o];  chunk b = rows 128b..128b+128
    W1r = w1.reshape(KH, F_IN, F1).reshape(KH * F_IN, F1)
    if use_b1:
        W1r = np.vstack([W1r, b1.reshape(F_IN, F1),
                         np.zeros((128 - F_IN, F1), np.float32)])
    W1re = np.concatenate([W1r[128 * b: 128 * (b + 1)] for b in range(nchunk)],
                          axis=1)                                   # [128, 64*nchunk]
    root_ext = np.vstack([root, ecc_bias[None, :]])                 # [33, F1]
    attn2 = np.stack([a_s, a_n], axis=1)                            # [F2, 2]
    gat_bias_rep = np.tile(gat_bias[None, :], (128, 1))             # [128, F2]
    pool_mask = np.zeros((128, NWIN), np.float32)
    for w in range(NWIN):
        v = np.arange(128) + w * WIN < NPC
        pool_mask[v, w] = 1.0
    shared = {
        "x_g": x,                                # gather table [N, F_IN]
        "w0m": np.ascontiguousarray(w0m),
        "W1re": np.ascontiguousarray(W1re),
        "root_ext": np.ascontiguousarray(root_ext),
        "gk": np.ascontiguousarray(gk),
        "attn2": np.ascontiguousarray(attn2),
        "gat_bias_rep": gat_bias_rep,
        "pool_mask": pool_mask,
        "fc_w": np.ascontiguousarray(fc_w),
        "fc_b": np.ascontiguousarray(fc_b.reshape(FC, 1)),
    }

    # --- per-core tensors ---
    in_maps = []
    srcT_all = (src // NPC) * NPC_PAD + (src % NPC)     # index into gathered T
    for c in range(NCORES):
        ids = eid[c]
        valid = ids >= 0
        idsv = np.where(valid, ids, 0)

        e_T = np.where(valid[None, :], e[idsv].T, 0.0).astype(np.float32)
        if use_b0:
            e_T = np.vstack([e_T, valid[None, :].astype(np.float32)])
        srcg = np.where(valid, src[idsv], 0).astype(np.int32)
        srcT = np.where(valid, srcT_all[idsv], 0).astype(np.int32)
        dstg = np.where(valid, dst_local[idsv], NPC_PAD).astype(np.int32)

        dl = np.where(valid, dst_local[idsv], -1)
        S = np.zeros((len(ids), 128), np.float32)
        t_of_slot = np.arange(len(ids)) // 128
        col = dl - win_of_tile[t_of_slot] * WIN
        ok = valid & (col >= 0) & (col < 128)
        S[np.nonzero(ok)[0], col[ok]] = 1.0

        x_T = np.zeros((F_IN + 1, NPC_PAD), np.float32)
        x_T[:F_IN, :NPC] = x[c * NPC:(c + 1) * NPC].T
        x_T[F_IN, :] = 1.0

        m = {
            "e_T": np.ascontiguousarray(e_T),
            "srcg": srcg.reshape(-1, 1),
            "srcT": srcT.reshape(-1, 1),
            "dstg": dstg.reshape(-1, 1),
            "S_d": np.ascontiguousarray(S),
            "x_T": x_T,
        }
        m.update(shared)
        in_maps.append(m)

    meta = dict(ke=ke, nchunk=nchunk, ntiles=ntiles, e_pad=e_pad,
                win_of_tile=win_of_tile, tiles_per_win=tiles_per_win)
    return in_maps, meta


def build_nc(meta):
    ke, nchunk = meta["ke"], meta["nchunk"]
    ntiles, e_pad = meta["ntiles"], meta["e_pad"]
    win_of_tile = meta["win_of_tile"]
    tiles_per_win = meta["tiles_per_win"]
    nsup = ntiles // ST

    nc = bacc.Bacc("TRN2", target_bir_lowering=False, debug=False,
                   enable_asserts=False, num_devices=NCORES)

    def din(name, shape, dt=F32):
        return nc.dram_tensor(name, shape, dt, kind="ExternalInput").ap()

    e_T = din("e_T", [ke, e_pad])
    srcg = din("srcg", [e_pad, 1], I32)
    srcT = din("srcT", [e_pad, 1], I32)
    dstg = din("dstg", [e_pad, 1], I32)
    S_d = din("S_d", [e_pad, 128])
    x_T = din("x_T", [F_IN + 1, NPC_PAD])
    x_g = din("x_g", [N, F_IN])
    w0m = din("w0m", [ke, KH])
    W1re = din("W1re", [128, F1 * nchunk])
    root_ext = din("root_ext", [F_IN + 1, F1])
    gk = din("gk", [F2, F2])
    attn2 = din("attn2", [F2, 2])
    gat_bias_rep = din("gat_bias_rep", [128, F2])
    pool_mask = din("pool_mask", [128, NWIN])
    fc_w = din("fc_w", [F2, FC])
    fc_b = din("fc_b", [FC, 1])
    out_d = nc.dram_tensor("out", [FC, 1], F32, kind="ExternalOutput").ap()

    # first/last tile flags per window
    first_of_win = {}
    tiles_of_win = [[] for _ in range(NWIN)]
    for t, w in enumerate(win_of_tile):
        tiles_of_win[int(w)].append(t)

    with tile.TileContext(nc) as tc:
        with (
            tc.tile_pool(name="res", bufs=1) as res,
            tc.tile_pool(name="dram", bufs=1, space="DRAM") as drp,
        ):
            # ---- resident SBUF tensors ----
            s_all = res.tile([128, ntiles * 128], F32)
            nc.sync.dma_start(
                s_all[:].rearrange("p (t n) -> p t n", n=128),
                S_d[:].rearrange("(t p) n -> p t n", p=128))
            xT_sb = res.tile([F_IN + 1, NPC_PAD], F32)
            nc.sync.dma_start(xT_sb[:], x_T[:])
            w0_sb = res.tile([ke, KH], F32)
            nc.sync.dma_start(w0_sb[:], w0m[:])
            W1_sb = res.tile([128, F1 * nchunk], F32)
            nc.sync.dma_start(W1_sb[:], W1re[:])
            root_sb = res.tile([F_IN + 1, F1], F32)
            nc.sync.dma_start(root_sb[:], root_ext[:])
            gk_sb = res.tile([F2, F2], F32)
            nc.sync.dma_start(gk_sb[:], gk[:])
            attn_sb = res.tile([F2, 2], F32)
            nc.sync.dma_start(attn_sb[:], attn2[:])
            gbias_sb = res.tile([128, F2], F32)
            nc.sync.dma_start(gbias_sb[:], gat_bias_rep[:])
            pmask_sb = res.tile([128, NWIN], F32)
            nc.sync.dma_start(pmask_sb[:], pool_mask[:])
            fcw_sb = res.tile([F2, FC], F32)
            nc.sync.dma_start(fcw_sb[:], fc_w[:])
            fcb_sb = res.tile([FC, 1], F32)
            nc.sync.dma_start(fcb_sb[:], fc_b[:])
            ident = res.tile([128, 128], F32)
            make_identity(nc, ident[:])
            x1_all = res.tile([128, NWIN * F1], F32)
            # resident per-edge index tiles: column t = indices of edge tile t
            src_res = res.tile([128, ntiles], I32)
            nc.sync.dma_start(src_res[:], srcg[:].rearrange("(t p) o -> p (t o)", p=128))
            srcT_res = res.tile([128, ntiles], I32)
            nc.sync.dma_start(srcT_res[:], srcT[:].rearrange("(t p) o -> p (t o)", p=128))
            dst_res = res.tile([128, ntiles], I32)
            nc.sync.dma_start(dst_res[:], dstg[:].rearrange("(t p) o -> p (t o)", p=128))

            # DRAM intermediates
            T_loc = drp.tile([NPC_PAD, F2 + 1], F32)
            T_full = drp.tile([NCORES * NPC_PAD, F2 + 1], F32)
            asf_col = drp.tile([NPC_PAD + 1, 1], F32)
            pool_in = drp.tile([F2, 1], F32)
            pool_out = drp.tile([F2, 1], F32)

            # ============ Phase A: ECC edges -> x1 ============
            with (
                tc.tile_pool(name="pa_sb", bufs=3) as sa,
                tc.tile_pool(name="pa_big", bufs=2) as sbig,
                tc.tile_pool(name="pa_h", bufs=2, space="PSUM") as ph,
                tc.tile_pool(name="pa_zt", bufs=2, space="PSUM") as pzt,
                tc.tile_pool(name="pa_ms", bufs=2, space="PSUM") as pms,
                tc.tile_pool(name="pa_ag", bufs=2, space="PSUM") as pag,
            ):
                agg_ps = {}
                # super-tile loop for the edge-parallel pipeline
                for s in range(nsup):
                    t0 = s * ST
                    eT_t = sa.tile([ke, ST * 128], F32, tag="eT")
                    nc.sync.dma_start(eT_t[:], e_T[:, t0 * 128:(t0 + ST) * 128])
                    xg = sa.tile([128, ST * F_IN], F32, tag="xg")
                    for j in range(ST):
                        nc.gpsimd.indirect_dma_start(
                            out=xg[:, j * F_IN:(j + 1) * F_IN], out_offset=None,
                            in_=x_g[:],
                            in_offset=bass.IndirectOffsetOnAxis(
                                ap=src_res[:, t0 + j:t0 + j + 1], axis=0))
                    # h = relu(e @ w0m) for the whole super-tile
                    h_ps = ph.tile([128, ST * KH], F32, space="PSUM", tag="h")
                    for j in range(ST):
                        nc.tensor.matmul(out=h_ps[:, j * KH:(j + 1) * KH],
                                         lhsT=eT_t[:, j * 128:(j + 1) * 128],
                                         rhs=w0_sb[:], start=True, stop=True)
                    h_sb = sa.tile([128, ST * KH], F32, tag="h_sb")
                    nc.scalar.activation(h_sb[:], h_ps[:], mybir.ActivationFunctionType.Relu)
                    # z[p, (k,i)] = h[p,k]*xg[p,i] per tile j
                    z = sbig.tile([128, ST * KH * F_IN], F32, tag="z")
                    for j in range(ST):
                        zv = z[:, j * 1024:(j + 1) * 1024].rearrange("p (k i) -> p k i", k=KH)
                        nc.vector.tensor_tensor(
                            out=zv,
                            in0=h_sb[:, j * KH:(j + 1) * KH].unsqueeze(2).broadcast_to([128, KH, F_IN]),
                            in1=xg[:, j * F_IN:(j + 1) * F_IN].unsqueeze(1).broadcast_to([128, KH, F_IN]),
                            op=mybir.AluOpType.mult)
                    # transpose z -> zT (PE), copies split ACT/DVE
                    zT = sbig.tile([128, ST * KH * F_IN], F32, tag="zT")
                    for j in range(ST):
                        for half in range(2):
                            zt_ps = pzt.tile([128, 512], F32, space="PSUM", tag="zt")
                            for b in range(4):
                                bb = half * 4 + b
                                nc.tensor.transpose(
                                    out=zt_ps[:, b * 128:(b + 1) * 128],
                                    in_=z[:, j * 1024 + bb * 128: j * 1024 + (bb + 1) * 128],
                                    identity=ident[:])
                            dst_ap = zT[:, j * 1024 + half * 512: j * 1024 + (half + 1) * 512]
                            if half == 0:
                                nc.scalar.activation(dst_ap, zt_ps[:],
                                                     mybir.ActivationFunctionType.Copy)
                            else:
                                nc.vector.tensor_copy(dst_ap, zt_ps[:])
                    if nchunk == 9:
                        # z chunk 8: [xg | zeros]; pairs with W1re rows [b1; 0]
                        z9 = sa.tile([128, ST * 128], F32, tag="z9")
                        nc.vector.memset(z9[:], 0.0)
                        for j in range(ST):
                            zt_ps9 = pzt.tile([128, 512], F32, space="PSUM", tag="zt")
                            nc.tensor.transpose(
                                out=zt_ps9[:F_IN, :128],
                                in_=xg[:, j * F_IN:(j + 1) * F_IN],
                                identity=ident[:])
                            nc.vector.tensor_copy(z9[:F_IN, j * 128:(j + 1) * 128],
                                                  zt_ps9[:F_IN, :128])
                    # msgs matmuls
                    for j in range(ST):
                        t = t0 + j
                        msgs_ps = pms.tile([128, F1], F32, space="PSUM", tag="msgs")
                        for b in range(8):
                            nc.tensor.matmul(
                                out=msgs_ps[:],
                                lhsT=zT[:, j * 1024 + b * 128: j * 1024 + (b + 1) * 128],
                                rhs=W1_sb[:, b * F1:(b + 1) * F1],
                                start=(b == 0), stop=(b == nchunk - 1))
                        if nchunk == 9:
                            nc.tensor.matmul(
                                out=msgs_ps[:], lhsT=z9[:, j * 128:(j + 1) * 128],
                                rhs=W1_sb[:, 8 * F1: 9 * F1], start=False, stop=True)
                        msgs_sb = sa.tile([128, F1], F32, tag="msgs_sb")
                        nc.scalar.activation(msgs_sb[:], msgs_ps[:],
                                             mybir.ActivationFunctionType.Copy)
                        w = int(win_of_tile[t])
                        if w not in agg_ps:
                            agg_ps[w] = pag.tile([128, F1], F32, space="PSUM", tag="agg", name=f"agg_{w}")
                            nc.tensor.matmul(out=agg_ps[w][:],
                                             lhsT=xT_sb[:, w * WIN:(w + 1) * WIN],
                                             rhs=root_sb[:], start=True, stop=False)
                        nc.tensor.matmul(out=agg_ps[w][:],
                                         lhsT=s_all[:, t * 128:(t + 1) * 128],
                                         rhs=msgs_sb[:], start=False,
                                         stop=(t == tiles_of_win[w][-1]))
                        if t == tiles_of_win[w][-1]:
                            nc.scalar.activation(x1_all[:, w * F1:(w + 1) * F1],
                                                 agg_ps[w][:],
                                                 mybir.ActivationFunctionType.Relu)
                            del agg_ps[w]
                # windows with no edge tiles (shouldn't happen, but be safe)
                for w in range(NWIN):
                    if not tiles_of_win[w]:
                        ap = pag.tile([128, F1], F32, space="PSUM", tag="agg")
                        nc.tensor.matmul(out=ap[:], lhsT=xT_sb[:, w * WIN:(w + 1) * WIN],
                                         rhs=root_sb[:], start=True, stop=True)
                        nc.scalar.activation(x1_all[:, w * F1:(w + 1) * F1], ap[:],
                                             mybir.ActivationFunctionType.Relu)

            # ============ Phase A2: x1 -> xp, attention scalars, T table ============
            with (
                tc.tile_pool(name="b_sb", bufs=3) as sb2,
                tc.tile_pool(name="b_ps", bufs=1, space="PSUM") as ps2,
            ):
                for w in range(NWIN):
                    x1t_ps = ps2.tile([F1, 128], F32, space="PSUM", tag="x1t")
                    nc.tensor.transpose(out=x1t_ps[:], in_=x1_all[:, w * F1:(w + 1) * F1],
                                        identity=ident[:])
                    x1t_sb = sb2.tile([F1, 128], F32, tag="x1t_sb")
                    nc.vector.tensor_copy(x1t_sb[:], x1t_ps[:])
                    xpt_ps = ps2.tile([F2, 128], F32, space="PSUM", tag="xpt")
                    nc.tensor.matmul(out=xpt_ps[:], lhsT=gk_sb[:], rhs=x1t_sb[:],
                                     start=True, stop=True)
                    xpt_sb = sb2.tile([F2, 128], F32, tag="xpt_sb")
                    nc.scalar.activation(xpt_sb[:], xpt_ps[:],
                                         mybir.ActivationFunctionType.Copy)
                    a_ps = ps2.tile([2, 128], F32, space="PSUM", tag="a")
                    nc.tensor.matmul(out=a_ps[:], lhsT=attn_sb[:], rhs=xpt_sb[:],
                                     start=True, stop=True)
                    a_sb = sb2.tile([2, 128], F32, tag="a_sb")
                    nc.vector.tensor_copy(a_sb[:], a_ps[:])
                    xp_ps = ps2.tile([128, F2], F32, space="PSUM", tag="xp")
                    nc.tensor.transpose(out=xp_ps[:], in_=xpt_sb[:], identity=ident[:F2, :F2])
                    acol_ps = ps2.tile([128, 2], F32, space="PSUM", tag="acol")
                    nc.tensor.transpose(out=acol_ps[:], in_=a_sb[:], identity=ident[:2, :2])
                    Tt = sb2.tile([128, F2 + 1], F32, tag="Tt")
                    nc.vector.tensor_copy(Tt[:, :F2], xp_ps[:])
                    nc.vector.tensor_copy(Tt[:, F2:F2 + 1], acol_ps[:, 1:2])
                    asf_sb = sb2.tile([128, 1], F32, tag="asf")
                    nc.vector.tensor_copy(asf_sb[:], acol_ps[:, 0:1])
                    nc.sync.dma_start(T_loc[w * WIN:(w + 1) * WIN, :], Tt[:])
                    nc.sync.dma_start(asf_col[w * WIN:(w + 1) * WIN, :], asf_sb[:])
                zz = sb2.tile([1, 1], F32, tag="zz")
                nc.vector.memset(zz[:], 0.0)
                nc.sync.dma_start(asf_col[NPC_PAD:NPC_PAD + 1, :], zz[:])

            # ============ AllGather T ============
            nc.gpsimd.collective_compute(
                "AllGather", mybir.AluOpType.bypass,
                replica_groups=[list(range(NCORES))],
                ins=[T_loc.opt()], outs=[T_full.opt()])

            # ============ Phase C: GAT edges ============
            with (
                tc.tile_pool(name="c_sb", bufs=3) as sc,
                tc.tile_pool(name="c_o2", bufs=2, space="PSUM") as po2,
                tc.tile_pool(name="c_pool", bufs=1, space="PSUM") as ppl,
            ):
                pool_ps = ppl.tile([F2, 1], F32, space="PSUM", tag="pool")
                out2_ps = {}
                for s in range(nsup):
                    t0 = s * ST
                    Tg = sc.tile([128, ST * (F2 + 1)], F32, tag="Tg")
                    asd = sc.tile([128, ST], F32, tag="asd")
                    for j in range(ST):
                        nc.gpsimd.indirect_dma_start(
                            out=Tg[:, j * 65:(j + 1) * 65], out_offset=None,
                            in_=T_full[:],
                            in_offset=bass.IndirectOffsetOnAxis(
                                ap=srcT_res[:, t0 + j:t0 + j + 1], axis=0))
                        nc.gpsimd.indirect_dma_start(
                            out=asd[:, j:j + 1], out_offset=None,
                            in_=asf_col[:],
                            in_offset=bass.IndirectOffsetOnAxis(
                                ap=dst_res[:, t0 + j:t0 + j + 1], axis=0))
                    sc_t = sc.tile([128, ST], F32, tag="sc")
                    nc.vector.tensor_tensor(
                        out=sc_t[:].unsqueeze(2), in0=asd[:].unsqueeze(2),
                        in1=Tg[:].rearrange("p (t f) -> p t f", f=65)[:, :, F2:F2 + 1],
                        op=mybir.AluOpType.add)
                    ex_t = sc.tile([128, ST], F32, tag="ex")
                    # exp(leaky_relu(s)): lrelu = max(s, 0.2 s)
                    lr = sc.tile([128, ST], F32, tag="lr")
                    nc.vector.tensor_scalar(out=lr[:], in0=sc_t[:], scalar1=0.2,
                                            scalar2=None, op0=mybir.AluOpType.mult)
                    nc.vector.tensor_tensor(out=lr[:], in0=lr[:], in1=sc_t[:],
                                            op=mybir.AluOpType.max)
                    nc.scalar.activation(ex_t[:], lr[:], mybir.ActivationFunctionType.Exp)
                    wm = sc.tile([128, ST * 65], F32, tag="wm")
                    for j in range(ST):
                        nc.vector.tensor_scalar(out=wm[:, j * 65:j * 65 + F2],
                                                in0=Tg[:, j * 65:j * 65 + F2],
                                                scalar1=ex_t[:, j:j + 1], scalar2=None,
                                                op0=mybir.AluOpType.mult)
                    nc.vector.tensor_copy(
                        wm[:].rearrange("p (t f) -> p t f", f=65)[:, :, F2:F2 + 1],
                        ex_t[:].unsqueeze(2))
                    for j in range(ST):
                        t = t0 + j
                        w = int(win_of_tile[t])
                        if w not in out2_ps:
                            out2_ps[w] = po2.tile([128, F2 + 1], F32, space="PSUM", tag="o2", name=f"o2_{w}")
                        nc.tensor.matmul(out=out2_ps[w][:],
                                         lhsT=s_all[:, t * 128:(t + 1) * 128],
                                         rhs=wm[:, j * 65:(j + 1) * 65],
                                         start=(t == tiles_of_win[w][0]),
                                         stop=(t == tiles_of_win[w][-1]))
                        if t == tiles_of_win[w][-1]:
                            o2 = out2_ps.pop(w)
                            dn = sc.tile([128, 1], F32, tag="dn")
                            nc.vector.tensor_scalar(out=dn[:], in0=o2[:, F2:F2 + 1],
                                                    scalar1=1e-9, scalar2=None,
                                                    op0=mybir.AluOpType.add)
                            rcp = sc.tile([128, 1], F32, tag="rcp")
                            nc.vector.reciprocal(rcp[:], dn[:])
                            x2 = sc.tile([128, F2], F32, tag="x2")
                            nc.vector.tensor_scalar(out=x2[:], in0=o2[:, :F2],
                                                    scalar1=rcp[:, :1], scalar2=None,
                                                    op0=mybir.AluOpType.mult)
                            nc.vector.tensor_tensor(out=x2[:], in0=x2[:], in1=gbias_sb[:],
                                                    op=mybir.AluOpType.add)
                            nc.scalar.activation(x2[:], x2[:],
                                                 mybir.ActivationFunctionType.Relu)
                            nc.tensor.matmul(out=pool_ps[:], lhsT=x2[:],
                                             rhs=pmask_sb[:, w:w + 1],
                                             start=(w == 0), stop=(w == NWIN - 1))

                # ============ Phase D: pool -> AllReduce -> Dense ============
                pooled = sc.tile([F2, 1], F32, tag="pooled")
                nc.scalar.activation(pooled[:], pool_ps[:],
                                     mybir.ActivationFunctionType.Copy, scale=1.0 / N)
                nc.gpsimd.dma_start(pool_in[:], pooled[:])
                nc.gpsimd.collective_compute(
                    "AllReduce", mybir.AluOpType.add,
                    replica_groups=[list(range(NCORES))],
                    ins=[pool_in.opt()], outs=[pool_out.opt()])
                pooled2 = sc.tile([F2, 1], F32, tag="pooled2")
                nc.sync.dma_start(pooled2[:], pool_out[:])
                fc_ps = ppl.tile([FC, 1], F32, space="PSUM", tag="fc")
                nc.tensor.matmul(out=fc_ps[:], lhsT=fcw_sb[:], rhs=pooled2[:],
                                 start=True, stop=True)
                out_sb = sc.tile([FC, 1], F32, tag="out")
                nc.scalar.activation(out_sb[:], fc_ps[:],
                                     mybir.ActivationFunctionType.Relu, bias=fcb_sb[:, :1])
                nc.sync.dma_start(out_d[:], out_sb[:])

    nc.compile()
    return nc


def kernel(**inputs):
    in_maps, meta = _host_inputs(inputs)
    key = (meta["ke"], meta["nchunk"], meta["ntiles"])
    if key not in _CACHE:
        _CACHE[key] = build_nc(meta)
    nc = _CACHE[key]
    res = bass_utils.run_bass_kernel_spmd(nc, in_maps, core_ids=list(range(NCORES)))
    return res.results[0]["out"].reshape(FC).astype(np.float32)


# revision 20
# speedup vs baseline: 1.3263x; 1.3263x over previous
"""GNN encoder (ECCConv -> GATConv -> GlobalAvgPool -> Dense) on 8 trn2 NeuronCores.

Edge-parallel by destination node: core c owns nodes [c*6250,(c+1)*6250) and the
edges pointing into them, so all segment reductions are core-local (PE one-hot
matmuls accumulated in PSUM per 128-node window). Per-edge node features are
fetched with batched SWDGE dma_gather from per-core compacted tables. Cross-core
traffic: one AllGather of the per-node GAT table and one AllReduce of the pooled
vector. Heavy per-edge tensors run in bf16.
"""
import sys

for _p in ("/opt/trn_rl_repo", "/root/.axon_site/_ro/trn_rl_repo"):
    if _p not in sys.path:
        sys.path.append(_p)

import numpy as np
import ml_dtypes

import concourse.bass as bass
import concourse.bacc as bacc
import concourse.tile as tile
import concourse.mybir as mybir
import concourse.bass_utils as bass_utils
from concourse.masks import make_identity
from concourse.library_config import mlp

F32 = mybir.dt.float32
BF16 = mybir.dt.bfloat16
I16 = mybir.dt.int16
I8 = mybir.dt.int8
BF = ml_dtypes.bfloat16

N = 50000
E = 100000
F_IN = 32
F_E = 8
F1 = 64
F2 = 64
KH = 32
FC = 32
NCORES = 8
NPC = N // NCORES
WIN = 128
NWIN = (NPC + WIN - 1) // WIN   # 49
NPC_PAD = NWIN * WIN            # 6272
ST = 4                          # edge tiles per super-tile
TW = 66                         # gathered T row prefix: [xp(64) | a_neigh | 1]

_CACHE = {}


def _wrap_idx(a):
    """int16 index layout for dma_gather: [i%16, i//16], replicated to 128 rows."""
    ni = len(a)
    return np.tile(a.astype(np.int16).reshape(ni // 16, 16).T, (8, 1))


def _host_shard(edge_index):
    src = np.asarray(edge_index[0], np.int64)
    dst = np.asarray(edge_index[1], np.int64)
    core = dst // NPC
    dst_local = dst - core * NPC
    win = dst_local // WIN

    per_core_edges = []
    cnt = np.zeros((NCORES, NWIN), np.int64)
    for c in range(NCORES):
        ids = np.nonzero(core == c)[0]
        ids = ids[np.argsort(dst_local[ids], kind="stable")]
        per_core_edges.append(ids)
        cnt[c] = np.bincount(win[ids], minlength=NWIN)

    tiles_per_win = np.ceil(cnt / 128).astype(np.int64).max(axis=0)
    ntiles = int(tiles_per_win.sum())
    if ntiles % ST:
        tiles_per_win[-1] += ST - ntiles % ST
        ntiles = int(tiles_per_win.sum())
    e_pad = ntiles * 128

    win_of_tile = np.repeat(np.arange(NWIN), tiles_per_win)
    slot_base = np.concatenate([[0], np.cumsum(tiles_per_win * 128)])

    eid = np.full((NCORES, e_pad), -1, np.int64)
    for c in range(NCORES):
        ids = per_core_edges[c]
        w_ids = win[ids]
        for w in range(NWIN):
            wi = ids[w_ids == w]
            eid[c, slot_base[w]: slot_base[w] + len(wi)] = wi
    return eid, win_of_tile, ntiles, e_pad, src, dst, dst_local


def _host_inputs(inputs):
    x = np.asarray(inputs["x"], np.float32)
    e = np.asarray(inputs["e"], np.float32)
    eid, win_of_tile, ntiles, e_pad, src, dst, dst_local = _host_shard(
        inputs["edge_index"])
    nsup = ntiles // ST

    w0 = np.asarray(inputs["ecc_w0"], np.float32)
    b0 = np.asarray(inputs["ecc_b0"], np.float32)
    w1 = np.asarray(inputs["ecc_w1"], np.float32)
    b1 = np.asarray(inputs["ecc_b1"], np.float32)
    root = np.asarray(inputs["ecc_root"], np.float32)
    ecc_bias = np.asarray(inputs["ecc_bias"], np.float32)
    gk = np.asarray(inputs["gat_kernel"], np.float32)
    a_s = np.asarray(inputs["gat_attn_self"], np.float32)
    a_n = np.asarray(inputs["gat_attn_neigh"], np.float32)
    gat_bias = np.asarray(inputs["gat_bias"], np.float32)
    fc_w = np.asarray(inputs["fc_w"], np.float32)
    fc_b = np.asarray(inputs["fc_b"], np.float32)

    use_b0 = bool(np.any(b0))
    use_b1 = bool(np.any(b1))
    ke = F_E + 1 if use_b0 else F_E
    nchunk = 9 if use_b1 else 8

    w0m = np.vstack([w0, b0[None, :]]) if use_b0 else w0
    W1r = w1.reshape(KH, F_IN, F1).reshape(KH * F_IN, F1)
    if use_b1:
        W1r = np.vstack([W1r, b1.reshape(F_IN, F1),
                         np.zeros((128 - F_IN, F1), np.float32)])
    W1re = np.concatenate([W1r[128 * b: 128 * (b + 1)] for b in range(nchunk)],
                          axis=1)
    root_ext = np.vstack([root, ecc_bias[None, :]])
    attn2 = np.stack([a_s, a_n], axis=1)
    gat_bias_rep = np.tile(gat_bias[None, :], (128, 1))
    pool_mask = np.zeros((128, NWIN), np.float32)
    for w in range(NWIN):
        v = np.arange(128) + w * WIN < NPC
        pool_mask[v, w] = 1.0
    shared = {
        "w0m": np.ascontiguousarray(w0m.astype(BF)),
        "W1re": np.ascontiguousarray(W1re.astype(BF)),
        "root_ext": np.ascontiguousarray(root_ext),
        "gk": np.ascontiguousarray(gk),
        "attn2": np.ascontiguousarray(attn2),
        "gat_bias_rep": gat_bias_rep,
        "pool_mask": pool_mask,
        "fc_w": np.ascontiguousarray(fc_w),
        "fc_b": np.ascontiguousarray(fc_b.reshape(FC, 1)),
    }

    # per-core compacted x gather table (indices stay < 32768 for int16)
    x128 = np.zeros((N, 128), BF)
    x128[:, :F_IN] = x.astype(BF)

    srcT_all = (src // NPC) * NPC_PAD + (src % NPC)

    per_core = []
    uniq_list = []
    for c in range(NCORES):
        ids = eid[c]
        valid = ids >= 0
        idsv = np.where(valid, ids, 0)
        s_glob = np.where(valid, src[idsv], 0)
        uniq, inv = np.unique(s_glob, return_inverse=True)
        uniq_list.append(uniq)
        per_core.append((ids, valid, idsv, s_glob, inv))
    u_pad = int(np.ceil(max(len(u) for u in uniq_list) / 128) * 128)

    in_maps = []
    for c in range(NCORES):
        ids, valid, idsv, s_glob, inv = per_core[c]
        uniq = uniq_list[c]

        e_T = np.where(valid[None, :], e[idsv].T, 0.0)
        if use_b0:
            e_T = np.vstack([e_T, valid[None, :].astype(np.float32)])

        xg_idx = np.concatenate(
            [_wrap_idx(inv[s * 512:(s + 1) * 512]) for s in range(nsup)], axis=1)
        x_c = np.zeros((u_pad, 128), BF)
        x_c[:len(uniq)] = x128[uniq]

        srcT = np.where(valid, srcT_all[idsv], 0)
        t2_idx = np.concatenate(
            [_wrap_idx(srcT[s * 512:(s + 1) * 512] // 2) for s in range(nsup)], axis=1)
        parity = (srcT % 2).astype(np.int8).reshape(ntiles, 128).T    # [128, ntiles]

        dl = np.where(valid, dst_local[idsv], -1)
        t_of_slot = np.arange(e_pad) // 128
        col = dl - win_of_tile[t_of_slot] * WIN
        ok = valid & (col >= 0) & (col < 128)
        S = np.zeros((e_pad, 128), BF)
        S[np.nonzero(ok)[0], col[ok]] = 1.0
        ST_d = np.ascontiguousarray(
            S.reshape(ntiles, 128, 128).transpose(0, 2, 1).reshape(e_pad, 128))

        x_T = np.zeros((F_IN + 1, NPC_PAD), np.float32)
        x_T[:F_IN, :NPC] = x[c * NPC:(c + 1) * NPC].T
        x_T[F_IN, :] = 1.0

        m = {
            "e_T": np.ascontiguousarray(e_T.astype(BF)),
            "x_c": x_c,
            "xg_idx": np.ascontiguousarray(xg_idx),
            "t2_idx": np.ascontiguousarray(t2_idx),
            "parity": np.ascontiguousarray(parity),
            "S_d": np.ascontiguousarray(S),
            "ST_d": ST_d,
            "x_T": x_T,
        }
        m.update(shared)
        in_maps.append(m)

    meta = dict(ke=ke, nchunk=nchunk, ntiles=ntiles, e_pad=e_pad, u_pad=u_pad,
                win_of_tile=win_of_tile)
    return in_maps, meta


def build_nc(meta, use_collectives=True, num_devices=NCORES):
    ke, nchunk = meta["ke"], meta["nchunk"]
    ntiles, e_pad, u_pad = meta["ntiles"], meta["e_pad"], meta["u_pad"]
    win_of_tile = meta["win_of_tile"]
    nsup = ntiles // ST

    nc = bacc.Bacc("TRN2", target_bir_lowering=False, debug=False,
                   enable_asserts=False, num_devices=num_devices)

    def din(name, shape, dt=F32):
        return nc.dram_tensor(name, shape, dt, kind="ExternalInput").ap()

    e_T = din("e_T", [ke, e_pad], BF16)
    x_c = din("x_c", [u_pad, 128], BF16)
    xg_idx = din("xg_idx", [128, nsup * 32], I16)
    t2_idx = din("t2_idx", [128, nsup * 32], I16)
    parity = din("parity", [128, ntiles], I8)
    S_d = din("S_d", [e_pad, 128], BF16)
    ST_dd = din("ST_d", [e_pad, 128], BF16)
    x_T = din("x_T", [F_IN + 1, NPC_PAD])
    w0m = din("w0m", [ke, KH], BF16)
    W1re = din("W1re", [128, F1 * nchunk], BF16)
    root_ext = din("root_ext", [F_IN + 1, F1])
    gk = din("gk", [F2, F2])
    attn2 = din("attn2", [F2, 2])
    gat_bias_rep = din("gat_bias_rep", [128, F2])
    pool_mask = din("pool_mask", [128, NWIN])
    fc_w = din("fc_w", [F2, FC])
    fc_b = din("fc_b", [FC, 1])
    out_d = nc.dram_tensor("out", [FC, 1], F32, kind="ExternalOutput").ap()

    tiles_of_win = [[] for _ in range(NWIN)]
    for t, w in enumerate(win_of_tile):
        tiles_of_win[int(w)].append(t)

    with tile.TileContext(nc) as tc:
        nc.gpsimd.load_library(mlp)
        with (
            tc.tile_pool(name="res", bufs=1) as res,
            tc.tile_pool(name="dram", bufs=1, space="DRAM") as drp,
        ):
            s_all = res.tile([128, ntiles * 128], BF16)
            nc.sync.dma_start(
                s_all[:].rearrange("p (t n) -> p t n", n=128),
                S_d[:].rearrange("(t p) n -> p t n", p=128))
            st_all = res.tile([128, ntiles * 128], BF16)
            nc.sync.dma_start(
                st_all[:].rearrange("p (t n) -> p t n", n=128),
                ST_dd[:].rearrange("(t p) n -> p t n", p=128))
            xgi_sb = res.tile([128, nsup * 32], I16)
            nc.sync.dma_start(xgi_sb[:], xg_idx[:])
            t2i_sb = res.tile([128, nsup * 32], I16)
            nc.sync.dma_start(t2i_sb[:], t2_idx[:])
            par_sb = res.tile([128, ntiles], I8)
            nc.sync.dma_start(par_sb[:], parity[:])
            xT_sb = res.tile([F_IN + 1, NPC_PAD], F32)
            nc.sync.dma_start(xT_sb[:], x_T[:])
            w0_sb = res.tile([ke, KH], BF16)
            nc.sync.dma_start(w0_sb[:], w0m[:])
            W1_sb = res.tile([128, F1 * nchunk], BF16)
            nc.sync.dma_start(W1_sb[:], W1re[:])
            root_sb = res.tile([F_IN + 1, F1], F32)
            nc.sync.dma_start(root_sb[:], root_ext[:])
            gk_sb = res.tile([F2, F2], F32)
            nc.sync.dma_start(gk_sb[:], gk[:])
            attn_sb = res.tile([F2, 2], F32)
            nc.sync.dma_start(attn_sb[:], attn2[:])
            gbias_sb = res.tile([128, F2], F32)
            nc.sync.dma_start(gbias_sb[:], gat_bias_rep[:])
            pmask_sb = res.tile([128, NWIN], F32)
            nc.sync.dma_start(pmask_sb[:], pool_mask[:])
            fcw_sb = res.tile([F2, FC], F32)
            nc.sync.dma_start(fcw_sb[:], fc_w[:])
            fcb_sb = res.tile([FC, 1], F32)
            nc.sync.dma_start(fcb_sb[:], fc_b[:])
            ident = res.tile([128, 128], F32)
            make_identity(nc, ident[:])
            ident_bf = res.tile([128, 128], BF16)
            make_identity(nc, ident_bf[:])
            x1_all = res.tile([128, NWIN * F1], F32)
            aself_sb = res.tile([128, NWIN], BF16)
            scores_all = res.tile([128, ntiles], F32)
            ex_all = res.tile([128, ntiles], F32)
            TgM_all = res.tile([128, ntiles * TW], BF16)

            T_loc = drp.tile([NPC_PAD, 128], BF16)
            T_full = drp.tile([NCORES * NPC_PAD, 128], BF16)
            pool_in = drp.tile([F2, 1], F32)
            pool_out = drp.tile([F2, 1], F32)

            # ============ Phase A: ECC edges -> x1 ============
            with (
                tc.tile_pool(name="pa_sb", bufs=3) as sa,
                tc.tile_pool(name="pa_big", bufs=2) as sbig,
                tc.tile_pool(name="pa_h", bufs=2, space="PSUM") as ph,
                tc.tile_pool(name="pa_zt", bufs=2, space="PSUM") as pzt,
                tc.tile_pool(name="pa_ms", bufs=2, space="PSUM") as pms,
                tc.tile_pool(name="pa_ag", bufs=2, space="PSUM") as pag,
            ):
                agg_ps = {}
                for s in range(nsup):
                    t0 = s * ST
                    eT_t = sa.tile([ke, ST * 128], BF16, tag="eT")
                    nc.sync.dma_start(eT_t[:], e_T[:, t0 * 128:(t0 + ST) * 128])
                    xg = sa.tile([128, ST * 128], BF16, tag="xg")
                    nc.gpsimd.dma_gather(
                        out_ap=xg[:].rearrange("p (c e) -> p c e", e=128),
                        in_ap=x_c[:], idxs_ap=xgi_sb[:, s * 32:(s + 1) * 32],
                        num_idxs=ST * 128, num_idxs_reg=ST * 128, elem_size=128)
                    h_ps = ph.tile([128, ST * KH], F32, space="PSUM", tag="h")
                    for j in range(ST):
                        nc.tensor.matmul(out=h_ps[:, j * KH:(j + 1) * KH],
                                         lhsT=eT_t[:, j * 128:(j + 1) * 128],
                                         rhs=w0_sb[:], start=True, stop=True)
                    h_sb = sa.tile([128, ST * KH], BF16, tag="h_sb")
                    nc.scalar.activation(h_sb[:], h_ps[:], mybir.ActivationFunctionType.Relu)
                    z = sbig.tile([128, ST * KH * F_IN], BF16, tag="z")
                    for j in range(ST):
                        zv = z[:, j * 1024:(j + 1) * 1024].rearrange("p (k i) -> p k i", k=KH)
                        eng = nc.gpsimd if j == 3 else nc.vector
                        eng.tensor_tensor(
                            out=zv,
                            in0=h_sb[:, j * KH:(j + 1) * KH].unsqueeze(2).broadcast_to([128, KH, F_IN]),
                            in1=xg[:, j * 128:j * 128 + F_IN].unsqueeze(1).broadcast_to([128, KH, F_IN]),
                            op=mybir.AluOpType.mult)
                    zT = sbig.tile([128, ST * KH * F_IN], BF16, tag="zT")
                    for j in range(ST):
                        for half in range(2):
                            zt_ps = pzt.tile([128, 512], BF16, space="PSUM", tag="zt")
                            for b in range(4):
                                bb = half * 4 + b
                                nc.tensor.transpose(
                                    out=zt_ps[:, b * 128:(b + 1) * 128],
                                    in_=z[:, j * 1024 + bb * 128: j * 1024 + (bb + 1) * 128],
                                    identity=ident_bf[:])
                            dst_ap = zT[:, j * 1024 + half * 512: j * 1024 + (half + 1) * 512]
                            if half == 0:
                                nc.scalar.activation(dst_ap, zt_ps[:],
                                                     mybir.ActivationFunctionType.Copy)
                            else:
                                nc.vector.tensor_copy(dst_ap, zt_ps[:])
                    if nchunk == 9:
                        z9 = sa.tile([128, ST * 128], BF16, tag="z9")
                        nc.vector.memset(z9[:], 0.0)
                        for j in range(ST):
                            zt_ps9 = pzt.tile([128, 512], BF16, space="PSUM", tag="zt")
                            nc.tensor.transpose(
                                out=zt_ps9[:F_IN, :128],
                                in_=xg[:, j * 128:j * 128 + F_IN],
                                identity=ident_bf[:])
                            nc.vector.tensor_copy(z9[:F_IN, j * 128:(j + 1) * 128],
                                                  zt_ps9[:F_IN, :128])
                    for j in range(ST):
                        t = t0 + j
                        msgs_ps = pms.tile([128, F1], F32, space="PSUM", tag="msgs")
                        for b in range(8):
                            nc.tensor.matmul(
                                out=msgs_ps[:],
                                lhsT=zT[:, j * 1024 + b * 128: j * 1024 + (b + 1) * 128],
                                rhs=W1_sb[:, b * F1:(b + 1) * F1],
                                start=(b == 0), stop=(b == nchunk - 1))
                        if nchunk == 9:
                            nc.tensor.matmul(
                                out=msgs_ps[:], lhsT=z9[:, j * 128:(j + 1) * 128],
                                rhs=W1_sb[:, 8 * F1: 9 * F1], start=False, stop=True)
                        msgs_sb = sa.tile([128, F1], BF16, tag="msgs_sb")
                        nc.scalar.activation(msgs_sb[:], msgs_ps[:],
                                             mybir.ActivationFunctionType.Copy)
                        w = int(win_of_tile[t])
                        if w not in agg_ps:
                            agg_ps[w] = pag.tile([128, F1], F32, space="PSUM",
                                                 tag="agg", name=f"agg_{w}")
                            nc.tensor.matmul(out=agg_ps[w][:],
                                             lhsT=xT_sb[:, w * WIN:(w + 1) * WIN],
                                             rhs=root_sb[:], start=True, stop=False,
                                             skip_group_check=True)
                        nc.tensor.matmul(out=agg_ps[w][:],
                                         lhsT=s_all[:, t * 128:(t + 1) * 128],
                                         rhs=msgs_sb[:], start=False,
                                         stop=(t == tiles_of_win[w][-1]),
                                         skip_group_check=True)
                        if t == tiles_of_win[w][-1]:
                            nc.scalar.activation(x1_all[:, w * F1:(w + 1) * F1],
                                                 agg_ps[w][:],
                                                 mybir.ActivationFunctionType.Relu)
                            del agg_ps[w]
                for w in range(NWIN):
                    if not tiles_of_win[w]:
                        ap = pag.tile([128, F1], F32, space="PSUM", tag="agg",
                                      name=f"agg_e{w}")
                        nc.tensor.matmul(out=ap[:], lhsT=xT_sb[:, w * WIN:(w + 1) * WIN],
                                         rhs=root_sb[:], start=True, stop=True)
                        nc.scalar.activation(x1_all[:, w * F1:(w + 1) * F1], ap[:],
                                             mybir.ActivationFunctionType.Relu)

            # ============ Phase A2: x1 -> xp, attention, T table ============
            with (
                tc.tile_pool(name="b_sb", bufs=3) as sb2,
                tc.tile_pool(name="b_ps", bufs=1, space="PSUM") as ps2,
            ):
                for w in range(NWIN):
                    x1t_ps = ps2.tile([F1, 128], F32, space="PSUM", tag="x1t")
                    nc.tensor.transpose(out=x1t_ps[:], in_=x1_all[:, w * F1:(w + 1) * F1],
                                        identity=ident[:])
                    x1t_sb = sb2.tile([F1, 128], F32, tag="x1t_sb")
                    nc.vector.tensor_copy(x1t_sb[:], x1t_ps[:])
                    xpt_ps = ps2.tile([F2, 128], F32, space="PSUM", tag="xpt")
                    nc.tensor.matmul(out=xpt_ps[:], lhsT=gk_sb[:], rhs=x1t_sb[:],
                                     start=True, stop=True)
                    xpt_sb = sb2.tile([F2, 128], F32, tag="xpt_sb")
                    nc.scalar.activation(xpt_sb[:], xpt_ps[:],
                                         mybir.ActivationFunctionType.Copy)
                    a_ps = ps2.tile([2, 128], F32, space="PSUM", tag="a")
                    nc.tensor.matmul(out=a_ps[:], lhsT=attn_sb[:], rhs=xpt_sb[:],
                                     start=True, stop=True)
                    a_sb = sb2.tile([2, 128], F32, tag="a_sb")
                    nc.vector.tensor_copy(a_sb[:], a_ps[:])
                    xp_ps = ps2.tile([128, F2], F32, space="PSUM", tag="xp")
                    nc.tensor.transpose(out=xp_ps[:], in_=xpt_sb[:], identity=ident[:F2, :F2])
                    acol_ps = ps2.tile([128, 2], F32, space="PSUM", tag="acol")
                    nc.tensor.transpose(out=acol_ps[:], in_=a_sb[:], identity=ident[:2, :2])
                    Tt = sb2.tile([128, 128], BF16, tag="Tt")
                    nc.vector.tensor_copy(Tt[:, :F2], xp_ps[:])
                    nc.vector.tensor_copy(Tt[:, F2:F2 + 1], acol_ps[:, 1:2])
                    nc.vector.memset(Tt[:, F2 + 1:F2 + 2], 1.0)
                    nc.vector.memset(Tt[:, F2 + 2:], 0.0)
                    nc.vector.tensor_copy(aself_sb[:, w:w + 1], acol_ps[:, 0:1])
                    nc.sync.dma_start(T_loc[w * WIN:(w + 1) * WIN, :], Tt[:])

            # ============ AllGather T ============
            if use_collectives:
                nc.gpsimd.collective_compute(
                    "AllGather", mybir.AluOpType.bypass,
                    replica_groups=[list(range(NCORES))],
                    ins=[T_loc.opt()], outs=[T_full.opt()])
            else:
                nc.sync.dma_start(T_full[:NPC_PAD, :], T_loc[:])

            T2 = T_full[:].rearrange("(v two) f -> v (two f)", two=2)

            # ============ Phase C: GAT ============
            with (
                tc.tile_pool(name="c_sb", bufs=3) as sc,
                tc.tile_pool(name="c_as", bufs=2, space="PSUM") as pas,
                tc.tile_pool(name="c_o2", bufs=2, space="PSUM") as po2,
                tc.tile_pool(name="c_pool", bufs=1, space="PSUM") as ppl,
            ):
                # C1: gather + merge + scores
                for s in range(nsup):
                    t0 = s * ST
                    Tg = sc.tile([128, ST * 256], BF16, tag="Tg")
                    nc.gpsimd.dma_gather(
                        out_ap=Tg[:].rearrange("p (c e) -> p c e", e=256),
                        in_ap=T2, idxs_ap=t2i_sb[:, s * 32:(s + 1) * 32],
                        num_idxs=ST * 128, num_idxs_reg=ST * 128, elem_size=256)
                    Tg3 = Tg[:].rearrange("p (c e) -> p c e", e=256)
                    selv = TgM_all[:, t0 * TW:(t0 + ST) * TW].rearrange(
                        "p (t f) -> p t f", f=TW)
                    nc.vector.tensor_copy(selv, Tg3[:, :, :TW])
                    nc.vector.copy_predicated(
                        selv,
                        par_sb[:, t0:t0 + ST].unsqueeze(2).broadcast_to([128, ST, TW]),
                        Tg3[:, :, 128:128 + TW])
                    asd_ps = pas.tile([128, ST], F32, space="PSUM", tag="asd")
                    for j in range(ST):
                        t = t0 + j
                        w = int(win_of_tile[t])
                        nc.tensor.matmul(out=asd_ps[:, j:j + 1],
                                         lhsT=st_all[:, t * 128:(t + 1) * 128],
                                         rhs=aself_sb[:, w:w + 1],
                                         start=True, stop=True)
                    nc.vector.tensor_tensor(
                        out=scores_all[:, t0:t0 + ST].unsqueeze(2),
                        in0=asd_ps[:].unsqueeze(2),
                        in1=TgM_all[:, t0 * TW:(t0 + ST) * TW].rearrange(
                            "p (t f) -> p t f", f=TW)[:, :, F2:F2 + 1],
                        op=mybir.AluOpType.add)
                # leaky relu + exp, batched
                lr = sc.tile([128, ntiles], F32, tag="lr")
                nc.vector.tensor_scalar(out=lr[:], in0=scores_all[:], scalar1=0.2,
                                        scalar2=None, op0=mybir.AluOpType.mult)
                nc.vector.tensor_tensor(out=lr[:], in0=lr[:], in1=scores_all[:],
                                        op=mybir.AluOpType.max)
                nc.scalar.activation(ex_all[:], lr[:], mybir.ActivationFunctionType.Exp)

                # C2: weight + scatter + windows
                pool_ps = ppl.tile([F2, 1], F32, space="PSUM", tag="pool")
                out2_ps = {}
                for t in range(ntiles):
                    wm = sc.tile([128, TW], BF16, tag="wm")
                    nc.vector.tensor_scalar(out=wm[:], in0=TgM_all[:, t * TW:(t + 1) * TW],
                                            scalar1=ex_all[:, t:t + 1], scalar2=None,
                                            op0=mybir.AluOpType.mult)
                    w = int(win_of_tile[t])
                    if w not in out2_ps:
                        out2_ps[w] = po2.tile([128, TW], F32, space="PSUM", tag="o2",
                                              name=f"o2_{w}")
                    nc.tensor.matmul(out=out2_ps[w][:],
                                     lhsT=s_all[:, t * 128:(t + 1) * 128],
                                     rhs=wm[:],
                                     start=(t == tiles_of_win[w][0]),
                                     stop=(t == tiles_of_win[w][-1]))
                    if t == tiles_of_win[w][-1]:
                        o2 = out2_ps.pop(w)
                        dn = sc.tile([128, 1], F32, tag="dn")
                        nc.vector.tensor_scalar(out=dn[:], in0=o2[:, TW - 1:TW],
                                                scalar1=1e-9, scalar2=None,
                                                op0=mybir.AluOpType.add)
                        rcp = sc.tile([128, 1], F32, tag="rcp")
                        nc.vector.reciprocal(rcp[:], dn[:])
                        x2 = sc.tile([128, F2], F32, tag="x2")
                        nc.vector.tensor_scalar(out=x2[:], in0=o2[:, :F2],
                                                scalar1=rcp[:, :1], scalar2=None,
                                                op0=mybir.AluOpType.mult)
                        nc.vector.tensor_tensor(out=x2[:], in0=x2[:], in1=gbias_sb[:],
                                                op=mybir.AluOpType.add)
                        nc.scalar.activation(x2[:], x2[:],
                                             mybir.ActivationFunctionType.Relu)
                        nc.tensor.matmul(out=pool_ps[:], lhsT=x2[:],
                                         rhs=pmask_sb[:, w:w + 1],
                                         start=(w == 0), stop=(w == NWIN - 1))

                # ============ Phase D ============
                pooled = sc.tile([F2, 1], F32, tag="pooled")
                nc.scalar.activation(pooled[:], pool_ps[:],
                                     mybir.ActivationFunctionType.Copy, scale=1.0 / N)
                nc.gpsimd.dma_start(pool_in[:], pooled[:])
                if use_collectives:
                    nc.gpsimd.collective_compute(
                        "AllReduce", mybir.AluOpType.add,
                        replica_groups=[list(range(NCORES))],
                        ins=[pool_in.opt()], outs=[pool_out.opt()])
                else:
                    nc.sync.dma_start(pool_out[:], pool_in[:])
                pooled2 = sc.tile([F2, 1], F32, tag="pooled2")
                nc.sync.dma_start(pooled2[:], pool_out[:])
                fc_ps = ppl.tile([FC, 1], F32, space="PSUM", tag="fc")
                nc.tensor.matmul(out=fc_ps[:], lhsT=fcw_sb[:], rhs=pooled2[:],
                                 start=True, stop=True)
                out_sb = sc.tile([FC, 1], F32, tag="out")
                nc.scalar.activation(out_sb[:], fc_ps[:],
                                     mybir.ActivationFunctionType.Relu, bias=fcb_sb[:, :1])
                nc.sync.dma_start(out_d[:], out_sb[:])

    nc.compile()
    return nc


def kernel(**inputs):
    in_maps, meta = _host_inputs(inputs)
    key = (meta["ke"], meta["nchunk"], meta["ntiles"], meta["u_pad"])
    if key not in _CACHE:
        _CACHE[key] = build_nc(meta)
    nc = _CACHE[key]
    res = bass_utils.run_bass_kernel_spmd(nc, in_maps, core_ids=list(range(NCORES)))
    return res.results[0]["out"].reshape(FC).astype(np.float32)


# revision 26
# speedup vs baseline: 2.9838x; 2.2496x over previous
"""GNN encoder (ECCConv -> GATConv -> GlobalAvgPool -> Dense) on 8 trn2 NeuronCores.

Edge-parallel by destination node: core c owns nodes [c*6250,(c+1)*6250) and the
edges pointing into them, so all segment reductions are core-local (PE one-hot
matmuls accumulated in PSUM per 128-node window). Per-edge node features are
fetched with batched SWDGE dma_gather from per-core compacted tables. Cross-core
traffic: one AllGather of the per-node GAT table and one AllReduce of the pooled
vector. Heavy per-edge tensors run in bf16.
"""
import sys

for _p in ("/opt/trn_rl_repo", "/root/.axon_site/_ro/trn_rl_repo"):
    if _p not in sys.path:
        sys.path.append(_p)

import numpy as np
import ml_dtypes

import concourse.bass as bass
import concourse.bacc as bacc
import concourse.tile as tile
import concourse.mybir as mybir
import concourse.bass_utils as bass_utils
from concourse.masks import make_identity
from concourse.library_config import mlp

F32 = mybir.dt.float32
BF16 = mybir.dt.bfloat16
I16 = mybir.dt.int16
I8 = mybir.dt.int8
BF = ml_dtypes.bfloat16

N = 50000
E = 100000
F_IN = 32
F_E = 8
F1 = 64
F2 = 64
KH = 32
FC = 32
NCORES = 8
NPC = N // NCORES
WIN = 128
NWIN = (NPC + WIN - 1) // WIN   # 49
NPC_PAD = NWIN * WIN            # 6272
ST = 4                          # edge tiles per super-tile
TW = 66                         # gathered T row prefix: [xp(64) | a_neigh | 1]

_CACHE = {}


def _wrap_idx(a):
    """int16 index layout for dma_gather: [i%16, i//16], replicated to 128 rows."""
    ni = len(a)
    return np.tile(a.astype(np.int16).reshape(ni // 16, 16).T, (8, 1))


def _host_shard(edge_index):
    """Deal 128-node global windows to cores (balanced by edge-tile count),
    order each core's windows by tile count so per-slot tile counts equalize
    across cores (SPMD needs one structure)."""
    src = np.asarray(edge_index[0], np.int64)
    dst = np.asarray(edge_index[1], np.int64)
    gw = dst // WIN
    ng = (N + WIN - 1) // WIN
    cnt = np.bincount(gw, minlength=ng)
    tiles_g = np.ceil(cnt / 128).astype(np.int64)

    order = np.argsort(-tiles_g, kind="stable")
    core_tiles = np.zeros(NCORES, np.int64)
    core_wins = [[] for _ in range(NCORES)]
    for g in order:
        cands = [c for c in range(NCORES) if len(core_wins[c]) < NWIN]
        c = min(cands, key=lambda c: (core_tiles[c], len(core_wins[c])))
        core_wins[c].append(int(g))
        core_tiles[c] += tiles_g[g]
    gmap = np.full((NCORES, NWIN), -1, np.int64)
    for c in range(NCORES):
        gmap[c, :len(core_wins[c])] = core_wins[c]

    tiles_per_slot = np.zeros(NWIN, np.int64)
    for j in range(NWIN):
        gs = gmap[:, j]
        tiles_per_slot[j] = max(tiles_g[g] if g >= 0 else 0 for g in gs)
    ntiles = int(tiles_per_slot.sum())
    if ntiles % ST:
        tiles_per_slot[0] += ST - ntiles % ST
        ntiles = int(tiles_per_slot.sum())
    e_pad = ntiles * 128
    win_of_tile = np.repeat(np.arange(NWIN), tiles_per_slot)
    slot_base = np.concatenate([[0], np.cumsum(tiles_per_slot * 128)])

    core_of_win = np.full(ng, -1, np.int64)
    slot_of_win = np.full(ng, -1, np.int64)
    for c in range(NCORES):
        for j, g in enumerate(core_wins[c]):
            core_of_win[g] = c
            slot_of_win[g] = j

    e_core = core_of_win[gw]
    e_slot = slot_of_win[gw]
    eid = np.full((NCORES, e_pad), -1, np.int64)
    for c in range(NCORES):
        ids = np.nonzero(e_core == c)[0]
        ids = ids[np.lexsort((dst[ids], e_slot[ids]))]
        sl = e_slot[ids]
        for j in range(NWIN):
            wi = ids[sl == j]
            eid[c, slot_base[j]: slot_base[j] + len(wi)] = wi

    col_of_edge = dst - gw * WIN
    gw_src = src // WIN
    srcT_all = (core_of_win[gw_src] * NPC_PAD + slot_of_win[gw_src] * WIN
                + (src - gw_src * WIN))
    return (eid, win_of_tile, ntiles, e_pad, src, col_of_edge, srcT_all, gmap)


def _host_inputs(inputs):
    x = np.asarray(inputs["x"], np.float32)
    e = np.asarray(inputs["e"], np.float32)
    (eid, win_of_tile, ntiles, e_pad, src, col_of_edge, srcT_all,
     gmap) = _host_shard(inputs["edge_index"])
    nsup = ntiles // ST

    w0 = np.asarray(inputs["ecc_w0"], np.float32)
    b0 = np.asarray(inputs["ecc_b0"], np.float32)
    w1 = np.asarray(inputs["ecc_w1"], np.float32)
    b1 = np.asarray(inputs["ecc_b1"], np.float32)
    root = np.asarray(inputs["ecc_root"], np.float32)
    ecc_bias = np.asarray(inputs["ecc_bias"], np.float32)
    gk = np.asarray(inputs["gat_kernel"], np.float32)
    a_s = np.asarray(inputs["gat_attn_self"], np.float32)
    a_n = np.asarray(inputs["gat_attn_neigh"], np.float32)
    gat_bias = np.asarray(inputs["gat_bias"], np.float32)
    fc_w = np.asarray(inputs["fc_w"], np.float32)
    fc_b = np.asarray(inputs["fc_b"], np.float32)

    use_b0 = bool(np.any(b0))
    use_b1 = bool(np.any(b1))
    ke = F_E + 1 if use_b0 else F_E
    nchunk = 9 if use_b1 else 8

    w0m = np.vstack([w0, b0[None, :]]) if use_b0 else w0
    W1r = w1.reshape(KH, F_IN, F1).reshape(KH * F_IN, F1)
    if use_b1:
        W1r = np.vstack([W1r, b1.reshape(F_IN, F1),
                         np.zeros((128 - F_IN, F1), np.float32)])
    W1re = np.concatenate([W1r[128 * b: 128 * (b + 1)] for b in range(nchunk)],
                          axis=1)
    root_ext = np.vstack([root, ecc_bias[None, :]])
    attn2 = np.stack([a_s, a_n], axis=1)
    gat_bias_rep = np.tile(gat_bias[None, :], (128, 1))
    shared = {
        "w0m": np.ascontiguousarray(w0m.astype(BF)),
        "W1re": np.ascontiguousarray(W1re.astype(BF)),
        "root_ext": np.ascontiguousarray(root_ext),
        "gk": np.ascontiguousarray(gk),
        "attn2": np.ascontiguousarray(attn2),
        "gat_bias_rep": gat_bias_rep,
        "fc_w": np.ascontiguousarray(fc_w),
        "fc_b": np.ascontiguousarray(fc_b.reshape(FC, 1)),
    }

    # per-core compacted x gather table (indices stay < 32768 for int16)
    x128 = np.zeros((N, 128), BF)
    x128[:, :F_IN] = x.astype(BF)

    per_core = []
    uniq_list = []
    for c in range(NCORES):
        ids = eid[c]
        valid = ids >= 0
        idsv = np.where(valid, ids, 0)
        s_glob = np.where(valid, src[idsv], 0)
        uniq, inv = np.unique(s_glob, return_inverse=True)
        uniq_list.append(uniq)
        per_core.append((ids, valid, idsv, s_glob, inv))
    u_pad = int(np.ceil(max(len(u) for u in uniq_list) / 128) * 128)

    in_maps = []
    for c in range(NCORES):
        ids, valid, idsv, s_glob, inv = per_core[c]
        uniq = uniq_list[c]

        e_T = np.where(valid[None, :], e[idsv].T, 0.0)
        if use_b0:
            e_T = np.vstack([e_T, valid[None, :].astype(np.float32)])

        xg_idx = np.concatenate(
            [_wrap_idx(inv[s * 512:(s + 1) * 512]) for s in range(nsup)], axis=1)
        x_c = np.zeros((u_pad, 128), BF)
        x_c[:len(uniq)] = x128[uniq]

        srcT = np.where(valid, srcT_all[idsv], 0)
        t2_idx = np.concatenate(
            [_wrap_idx(srcT[s * 512:(s + 1) * 512] // 2) for s in range(nsup)], axis=1)
        parity = (srcT % 2).astype(np.int8).reshape(ntiles, 128).T    # [128, ntiles]

        col = np.where(valid, col_of_edge[idsv], -1)
        ok = valid & (col >= 0) & (col < 128)
        S = np.zeros((e_pad, 128), BF)
        S[np.nonzero(ok)[0], col[ok]] = 1.0
        ST_d = np.ascontiguousarray(
            S.reshape(ntiles, 128, 128).transpose(0, 2, 1).reshape(e_pad, 128))

        x_T = np.zeros((F_IN + 1, NPC_PAD), np.float32)
        pool_mask = np.zeros((128, NWIN), np.float32)
        for j in range(NWIN):
            g = gmap[c, j]
            if g < 0:
                continue
            lo = g * WIN
            hi = min(lo + WIN, N)
            x_T[:F_IN, j * WIN: j * WIN + hi - lo] = x[lo:hi].T
            pool_mask[:hi - lo, j] = 1.0
        x_T[F_IN, :] = 1.0

        m = {
            "e_T": np.ascontiguousarray(e_T.astype(BF)),
            "x_c": x_c,
            "xg_idx": np.ascontiguousarray(xg_idx),
            "t2_idx": np.ascontiguousarray(t2_idx),
            "parity": np.ascontiguousarray(parity),
            "S_d": np.ascontiguousarray(S),
            "ST_d": ST_d,
            "x_T": x_T,
            "pool_mask": pool_mask,
        }
        m.update(shared)
        in_maps.append(m)

    meta = dict(ke=ke, nchunk=nchunk, ntiles=ntiles, e_pad=e_pad, u_pad=u_pad,
                win_of_tile=win_of_tile)
    return in_maps, meta


def build_nc(meta, use_collectives=True, num_devices=NCORES):
    ke, nchunk = meta["ke"], meta["nchunk"]
    ntiles, e_pad, u_pad = meta["ntiles"], meta["e_pad"], meta["u_pad"]
    win_of_tile = meta["win_of_tile"]
    nsup = ntiles // ST

    nc = bacc.Bacc("TRN2", target_bir_lowering=False, debug=False,
                   enable_asserts=False, num_devices=num_devices)

    def din(name, shape, dt=F32):
        return nc.dram_tensor(name, shape, dt, kind="ExternalInput").ap()

    e_T = din("e_T", [ke, e_pad], BF16)
    x_c = din("x_c", [u_pad, 128], BF16)
    xg_idx = din("xg_idx", [128, nsup * 32], I16)
    t2_idx = din("t2_idx", [128, nsup * 32], I16)
    parity = din("parity", [128, ntiles], I8)
    S_d = din("S_d", [e_pad, 128], BF16)
    ST_dd = din("ST_d", [e_pad, 128], BF16)
    x_T = din("x_T", [F_IN + 1, NPC_PAD])
    w0m = din("w0m", [ke, KH], BF16)
    W1re = din("W1re", [128, F1 * nchunk], BF16)
    root_ext = din("root_ext", [F_IN + 1, F1])
    gk = din("gk", [F2, F2])
    attn2 = din("attn2", [F2, 2])
    gat_bias_rep = din("gat_bias_rep", [128, F2])
    pool_mask = din("pool_mask", [128, NWIN])
    fc_w = din("fc_w", [F2, FC])
    fc_b = din("fc_b", [FC, 1])
    out_d = nc.dram_tensor("out", [FC, 1], F32, kind="ExternalOutput").ap()

    tiles_of_win = [[] for _ in range(NWIN)]
    for t, w in enumerate(win_of_tile):
        tiles_of_win[int(w)].append(t)

    with tile.TileContext(nc) as tc:
        nc.gpsimd.load_library(mlp)
        with (
            tc.tile_pool(name="res", bufs=1) as res,
            tc.tile_pool(name="dram", bufs=1, space="DRAM") as drp,
        ):
            s_all = res.tile([128, ntiles * 128], BF16)
            nc.sync.dma_start(
                s_all[:].rearrange("p (t n) -> p t n", n=128),
                S_d[:].rearrange("(t p) n -> p t n", p=128))
            st_all = res.tile([128, ntiles * 128], BF16)
            nc.sync.dma_start(
                st_all[:].rearrange("p (t n) -> p t n", n=128),
                ST_dd[:].rearrange("(t p) n -> p t n", p=128))
            xgi_sb = res.tile([128, nsup * 32], I16)
            nc.sync.dma_start(xgi_sb[:], xg_idx[:])
            t2i_sb = res.tile([128, nsup * 32], I16)
            nc.sync.dma_start(t2i_sb[:], t2_idx[:])
            par_sb = res.tile([128, ntiles], I8)
            nc.sync.dma_start(par_sb[:], parity[:])
            xT_sb = res.tile([F_IN + 1, NPC_PAD], F32)
            nc.sync.dma_start(xT_sb[:], x_T[:])
            w0_sb = res.tile([ke, KH], BF16)
            nc.sync.dma_start(w0_sb[:], w0m[:])
            W1_sb = res.tile([128, F1 * nchunk], BF16)
            nc.sync.dma_start(W1_sb[:], W1re[:])
            root_sb = res.tile([F_IN + 1, F1], F32)
            nc.sync.dma_start(root_sb[:], root_ext[:])
            gk_sb = res.tile([F2, F2], F32)
            nc.sync.dma_start(gk_sb[:], gk[:])
            attn_sb = res.tile([F2, 2], F32)
            nc.sync.dma_start(attn_sb[:], attn2[:])
            gbias_sb = res.tile([128, F2], F32)
            nc.sync.dma_start(gbias_sb[:], gat_bias_rep[:])
            pmask_sb = res.tile([128, NWIN], F32)
            nc.sync.dma_start(pmask_sb[:], pool_mask[:])
            fcw_sb = res.tile([F2, FC], F32)
            nc.sync.dma_start(fcw_sb[:], fc_w[:])
            fcb_sb = res.tile([FC, 1], F32)
            nc.sync.dma_start(fcb_sb[:], fc_b[:])
            ident = res.tile([128, 128], F32)
            make_identity(nc, ident[:])
            ident_bf = res.tile([128, 128], BF16)
            make_identity(nc, ident_bf[:])
            x1_all = res.tile([128, NWIN * F1], F32)
            aself_sb = res.tile([128, NWIN], BF16)
            scores_all = res.tile([128, ntiles], F32)
            ex_all = res.tile([128, ntiles], F32)
            TgM_all = res.tile([128, ntiles * TW], BF16)

            T_loc = drp.tile([NPC_PAD, 128], BF16)
            T_full = drp.tile([NCORES * NPC_PAD, 128], BF16)
            pool_in = drp.tile([F2, 1], F32)
            pool_out = drp.tile([F2, 1], F32)

            # ============ Phase A: ECC edges -> x1 ============
            with (
                tc.tile_pool(name="pa_sb", bufs=3) as sa,
                tc.tile_pool(name="pa_big", bufs=2) as sbig,
                tc.tile_pool(name="pa_h", bufs=1, space="PSUM") as ph,
                tc.tile_pool(name="pa_zt", bufs=4, space="PSUM") as pzt,
                tc.tile_pool(name="pa_ms", bufs=1, space="PSUM") as pms,
                tc.tile_pool(name="pa_ag", bufs=2, space="PSUM") as pag,
            ):
                agg_ps = {}
                for s in range(nsup):
                    t0 = s * ST
                    eT_t = sa.tile([ke, ST * 128], BF16, tag="eT")
                    nc.sync.dma_start(eT_t[:], e_T[:, t0 * 128:(t0 + ST) * 128])
                    xg = sa.tile([128, ST * 128], BF16, tag="xg")
                    nc.gpsimd.dma_gather(
                        out_ap=xg[:].rearrange("p (c e) -> p c e", e=128),
                        in_ap=x_c[:], idxs_ap=xgi_sb[:, s * 32:(s + 1) * 32],
                        num_idxs=ST * 128, num_idxs_reg=ST * 128, elem_size=128)
                    h_ps = ph.tile([128, ST * KH], F32, space="PSUM", tag="h")
                    for j in range(ST):
                        nc.tensor.matmul(out=h_ps[:, j * KH:(j + 1) * KH],
                                         lhsT=eT_t[:, j * 128:(j + 1) * 128],
                                         rhs=w0_sb[:], start=True, stop=True)
                    h_sb = sa.tile([128, ST * KH], BF16, tag="h_sb")
                    nc.scalar.activation(h_sb[:], h_ps[:], mybir.ActivationFunctionType.Relu)
                    z = sbig.tile([128, ST * KH * F_IN], BF16, tag="z")
                    for j in range(ST):
                        zv = z[:, j * 1024:(j + 1) * 1024].rearrange("p (k i) -> p k i", k=KH)
                        eng = nc.gpsimd if j == 3 else nc.vector
                        eng.tensor_tensor(
                            out=zv,
                            in0=h_sb[:, j * KH:(j + 1) * KH].unsqueeze(2).broadcast_to([128, KH, F_IN]),
                            in1=xg[:, j * 128:j * 128 + F_IN].unsqueeze(1).broadcast_to([128, KH, F_IN]),
                            op=mybir.AluOpType.mult)
                    zT = sbig.tile([128, ST * KH * F_IN], BF16, tag="zT")
                    for j in range(ST):
                        for half in range(2):
                            zt_ps = pzt.tile([128, 512], BF16, space="PSUM", tag="zt")
                            for b in range(4):
                                bb = half * 4 + b
                                nc.tensor.transpose(
                                    out=zt_ps[:, b * 128:(b + 1) * 128],
                                    in_=z[:, j * 1024 + bb * 128: j * 1024 + (bb + 1) * 128],
                                    identity=ident_bf[:])
                            dst_ap = zT[:, j * 1024 + half * 512: j * 1024 + (half + 1) * 512]
                            if (j * 2 + half) % 8 < 5:
                                nc.scalar.activation(dst_ap, zt_ps[:],
                                                     mybir.ActivationFunctionType.Copy)
                            else:
                                nc.vector.tensor_copy(dst_ap, zt_ps[:])
                    if nchunk == 9:
                        z9 = sa.tile([128, ST * 128], BF16, tag="z9")
                        nc.vector.memset(z9[:], 0.0)
                        for j in range(ST):
                            zt_ps9 = pzt.tile([128, 512], BF16, space="PSUM", tag="zt")
                            nc.tensor.transpose(
                                out=zt_ps9[:F_IN, :128],
                                in_=xg[:, j * 128:j * 128 + F_IN],
                                identity=ident_bf[:])
                            nc.vector.tensor_copy(z9[:F_IN, j * 128:(j + 1) * 128],
                                                  zt_ps9[:F_IN, :128])
                    msgs_ps = pms.tile([128, ST * F1], F32, space="PSUM", tag="msgs")
                    for j in range(ST):
                        for b in range(8):
                            nc.tensor.matmul(
                                out=msgs_ps[:, j * F1:(j + 1) * F1],
                                lhsT=zT[:, j * 1024 + b * 128: j * 1024 + (b + 1) * 128],
                                rhs=W1_sb[:, b * F1:(b + 1) * F1],
                                start=(b == 0), stop=(b == nchunk - 1))
                        if nchunk == 9:
                            nc.tensor.matmul(
                                out=msgs_ps[:, j * F1:(j + 1) * F1],
                                lhsT=z9[:, j * 128:(j + 1) * 128],
                                rhs=W1_sb[:, 8 * F1: 9 * F1], start=False, stop=True)
                    msgs_all = sa.tile([128, ST * F1], BF16, tag="msgs_sb")
                    nc.scalar.activation(msgs_all[:], msgs_ps[:],
                                         mybir.ActivationFunctionType.Copy)
                    for j in range(ST):
                        t = t0 + j
                        msgs_sb = msgs_all[:, j * F1:(j + 1) * F1]
                        w = int(win_of_tile[t])
                        if w not in agg_ps:
                            agg_ps[w] = pag.tile([128, F1], F32, space="PSUM",
                                                 tag="agg", name=f"agg_{w}")
                            nc.tensor.matmul(out=agg_ps[w][:],
                                             lhsT=xT_sb[:, w * WIN:(w + 1) * WIN],
                                             rhs=root_sb[:], start=True, stop=False,
                                             skip_group_check=True)
                        nc.tensor.matmul(out=agg_ps[w][:],
                                         lhsT=s_all[:, t * 128:(t + 1) * 128],
                                         rhs=msgs_sb, start=False,
                                         stop=(t == tiles_of_win[w][-1]),
                                         skip_group_check=True)
                        if t == tiles_of_win[w][-1]:
                            nc.scalar.activation(x1_all[:, w * F1:(w + 1) * F1],
                                                 agg_ps[w][:],
                                                 mybir.ActivationFunctionType.Relu)
                            del agg_ps[w]
                for w in range(NWIN):
                    if not tiles_of_win[w]:
                        ap = pag.tile([128, F1], F32, space="PSUM", tag="agg",
                                      name=f"agg_e{w}")
                        nc.tensor.matmul(out=ap[:], lhsT=xT_sb[:, w * WIN:(w + 1) * WIN],
                                         rhs=root_sb[:], start=True, stop=True)
                        nc.scalar.activation(x1_all[:, w * F1:(w + 1) * F1], ap[:],
                                             mybir.ActivationFunctionType.Relu)

            # ============ Phase A2: x1 -> xp, attention, T table ============
            with (
                tc.tile_pool(name="b_sb", bufs=3) as sb2,
                tc.tile_pool(name="b_ps", bufs=1, space="PSUM") as ps2,
            ):
                for w in range(NWIN):
                    x1t_ps = ps2.tile([F1, 128], F32, space="PSUM", tag="x1t")
                    nc.tensor.transpose(out=x1t_ps[:], in_=x1_all[:, w * F1:(w + 1) * F1],
                                        identity=ident[:])
                    x1t_sb = sb2.tile([F1, 128], F32, tag="x1t_sb")
                    nc.vector.tensor_copy(x1t_sb[:], x1t_ps[:])
                    xpt_ps = ps2.tile([F2, 128], F32, space="PSUM", tag="xpt")
                    nc.tensor.matmul(out=xpt_ps[:], lhsT=gk_sb[:], rhs=x1t_sb[:],
                                     start=True, stop=True)
                    xpt_sb = sb2.tile([F2, 128], F32, tag="xpt_sb")
                    nc.scalar.activation(xpt_sb[:], xpt_ps[:],
                                         mybir.ActivationFunctionType.Copy)
                    a_ps = ps2.tile([2, 128], F32, space="PSUM", tag="a")
                    nc.tensor.matmul(out=a_ps[:], lhsT=attn_sb[:], rhs=xpt_sb[:],
                                     start=True, stop=True)
                    a_sb = sb2.tile([2, 128], F32, tag="a_sb")
                    nc.vector.tensor_copy(a_sb[:], a_ps[:])
                    xp_ps = ps2.tile([128, F2], F32, space="PSUM", tag="xp")
                    nc.tensor.transpose(out=xp_ps[:], in_=xpt_sb[:], identity=ident[:F2, :F2])
                    acol_ps = ps2.tile([128, 2], F32, space="PSUM", tag="acol")
                    nc.tensor.transpose(out=acol_ps[:], in_=a_sb[:], identity=ident[:2, :2])
                    Tt = sb2.tile([128, 128], BF16, tag="Tt")
                    nc.vector.tensor_copy(Tt[:, :F2], xp_ps[:])
                    nc.vector.tensor_copy(Tt[:, F2:F2 + 1], acol_ps[:, 1:2])
                    nc.vector.memset(Tt[:, F2 + 1:F2 + 2], 1.0)
                    nc.vector.memset(Tt[:, F2 + 2:], 0.0)
                    nc.vector.tensor_copy(aself_sb[:, w:w + 1], acol_ps[:, 0:1])
                    nc.sync.dma_start(T_loc[w * WIN:(w + 1) * WIN, :], Tt[:])

            # ============ AllGather T ============
            if use_collectives:
                nc.gpsimd.collective_compute(
                    "AllGather", mybir.AluOpType.bypass,
                    replica_groups=[list(range(NCORES))],
                    ins=[T_loc.opt()], outs=[T_full.opt()])
            else:
                nc.sync.dma_start(T_full[:NPC_PAD, :], T_loc[:])

            T2 = T_full[:].rearrange("(v two) f -> v (two f)", two=2)

            # ============ Phase C: GAT ============
            with (
                tc.tile_pool(name="c_sb", bufs=3) as sc,
                tc.tile_pool(name="c_as", bufs=2, space="PSUM") as pas,
                tc.tile_pool(name="c_o2", bufs=2, space="PSUM") as po2,
                tc.tile_pool(name="c_pool", bufs=1, space="PSUM") as ppl,
            ):
                # C1: gather + merge + scores
                for s in range(nsup):
                    t0 = s * ST
                    Tg = sc.tile([128, ST * 256], BF16, tag="Tg")
                    nc.gpsimd.dma_gather(
                        out_ap=Tg[:].rearrange("p (c e) -> p c e", e=256),
                        in_ap=T2, idxs_ap=t2i_sb[:, s * 32:(s + 1) * 32],
                        num_idxs=ST * 128, num_idxs_reg=ST * 128, elem_size=256)
                    Tg3 = Tg[:].rearrange("p (c e) -> p c e", e=256)
                    selv = TgM_all[:, t0 * TW:(t0 + ST) * TW].rearrange(
                        "p (t f) -> p t f", f=TW)
                    nc.vector.tensor_copy(selv, Tg3[:, :, :TW])
                    nc.vector.copy_predicated(
                        selv,
                        par_sb[:, t0:t0 + ST].unsqueeze(2).broadcast_to([128, ST, TW]),
                        Tg3[:, :, 128:128 + TW])
                    asd_ps = pas.tile([128, ST], F32, space="PSUM", tag="asd")
                    for j in range(ST):
                        t = t0 + j
                        w = int(win_of_tile[t])
                        nc.tensor.matmul(out=asd_ps[:, j:j + 1],
                                         lhsT=st_all[:, t * 128:(t + 1) * 128],
                                         rhs=aself_sb[:, w:w + 1],
                                         start=True, stop=True)
                    nc.vector.tensor_tensor(
                        out=scores_all[:, t0:t0 + ST].unsqueeze(2),
                        in0=asd_ps[:].unsqueeze(2),
                        in1=TgM_all[:, t0 * TW:(t0 + ST) * TW].rearrange(
                            "p (t f) -> p t f", f=TW)[:, :, F2:F2 + 1],
                        op=mybir.AluOpType.add)
                # leaky relu + exp, batched
                lr = sc.tile([128, ntiles], F32, tag="lr")
                nc.vector.tensor_scalar(out=lr[:], in0=scores_all[:], scalar1=0.2,
                                        scalar2=None, op0=mybir.AluOpType.mult)
                nc.vector.tensor_tensor(out=lr[:], in0=lr[:], in1=scores_all[:],
                                        op=mybir.AluOpType.max)
                nc.scalar.activation(ex_all[:], lr[:], mybir.ActivationFunctionType.Exp)

                # C2: weight + scatter + windows
                pool_ps = ppl.tile([F2, 1], F32, space="PSUM", tag="pool")
                out2_ps = {}
                for t in range(ntiles):
                    wm = sc.tile([128, TW], BF16, tag="wm")
                    if t % 2 == 0:
                        nc.vector.tensor_scalar(out=wm[:], in0=TgM_all[:, t * TW:(t + 1) * TW],
                                                scalar1=ex_all[:, t:t + 1], scalar2=None,
                                                op0=mybir.AluOpType.mult)
                    else:
                        nc.scalar.activation(wm[:], TgM_all[:, t * TW:(t + 1) * TW],
                                             mybir.ActivationFunctionType.Copy,
                                             scale=ex_all[:, t:t + 1])
                    w = int(win_of_tile[t])
                    if w not in out2_ps:
                        out2_ps[w] = po2.tile([128, TW], F32, space="PSUM", tag="o2",
                                              name=f"o2_{w}")
                    nc.tensor.matmul(out=out2_ps[w][:],
                                     lhsT=s_all[:, t * 128:(t + 1) * 128],
                                     rhs=wm[:],
                                     start=(t == tiles_of_win[w][0]),
                                     stop=(t == tiles_of_win[w][-1]))
                    if t == tiles_of_win[w][-1]:
                        o2 = out2_ps.pop(w)
                        dn = sc.tile([128, 1], F32, tag="dn")
                        nc.vector.tensor_scalar(out=dn[:], in0=o2[:, TW - 1:TW],
                                                scalar1=1e-9, scalar2=None,
                                                op0=mybir.AluOpType.add)
                        rcp = sc.tile([128, 1], F32, tag="rcp")
                        nc.vector.reciprocal(rcp[:], dn[:])
                        x2 = sc.tile([128, F2], F32, tag="x2")
                        nc.vector.tensor_scalar(out=x2[:], in0=o2[:, :F2],
                                                scalar1=rcp[:, :1], scalar2=None,
                                                op0=mybir.AluOpType.mult)
                        nc.vector.tensor_tensor(out=x2[:], in0=x2[:], in1=gbias_sb[:],
                                                op=mybir.AluOpType.add)
                        nc.scalar.activation(x2[:], x2[:],
                                             mybir.ActivationFunctionType.Relu)
                        nc.tensor.matmul(out=pool_ps[:], lhsT=x2[:],
                                         rhs=pmask_sb[:, w:w + 1],
                                         start=(w == 0), stop=(w == NWIN - 1))

                # ============ Phase D ============
                pooled = sc.tile([F2, 1], F32, tag="pooled")
                nc.scalar.activation(pooled[:], pool_ps[:],
                                     mybir.ActivationFunctionType.Copy, scale=1.0 / N)
                nc.gpsimd.dma_start(pool_in[:], pooled[:])
                if use_collectives:
                    nc.gpsimd.collective_compute(
                        "AllReduce", mybir.AluOpType.add,
                        replica_groups=[list(range(NCORES))],
                        ins=[pool_in.opt()], outs=[pool_out.opt()])
                else:
                    nc.sync.dma_start(pool_out[:], pool_in[:])
                pooled2 = sc.tile([F2, 1], F32, tag="pooled2")
                nc.sync.dma_start(pooled2[:], pool_out[:])
                fc_ps = ppl.tile([FC, 1], F32, space="PSUM", tag="fc")
                nc.tensor.matmul(out=fc_ps[:], lhsT=fcw_sb[:], rhs=pooled2[:],
                                 start=True, stop=True)
                out_sb = sc.tile([FC, 1], F32, tag="out")
                nc.scalar.activation(out_sb[:], fc_ps[:],
                                     mybir.ActivationFunctionType.Relu, bias=fcb_sb[:, :1])
                nc.sync.dma_start(out_d[:], out_sb[:])

    nc.compile()
    return nc


def kernel(**inputs):
    in_maps, meta = _host_inputs(inputs)
    key = (meta["ke"], meta["nchunk"], meta["ntiles"], meta["u_pad"])
    if key not in _CACHE:
        _CACHE[key] = build_nc(meta)
    nc = _CACHE[key]
    res = bass_utils.run_bass_kernel_spmd(nc, in_maps, core_ids=list(range(NCORES)))
    return res.results[0]["out"].reshape(FC).astype(np.float32)


# revision 31
# speedup vs baseline: 3.0296x; 1.0154x over previous
"""GNN encoder (ECCConv -> GATConv -> GlobalAvgPool -> Dense) on 8 trn2 NeuronCores.

Edge-parallel by destination node: core c owns nodes [c*6250,(c+1)*6250) and the
edges pointing into them, so all segment reductions are core-local (PE one-hot
matmuls accumulated in PSUM per 128-node window). Per-edge node features are
fetched with batched SWDGE dma_gather from per-core compacted tables. Cross-core
traffic: one AllGather of the per-node GAT table and one AllReduce of the pooled
vector. Heavy per-edge tensors run in bf16.
"""
import sys

for _p in ("/opt/trn_rl_repo", "/root/.axon_site/_ro/trn_rl_repo"):
    if _p not in sys.path:
        sys.path.append(_p)

import numpy as np
import ml_dtypes

import concourse.bass as bass
import concourse.bacc as bacc
import concourse.tile as tile
import concourse.mybir as mybir
import concourse.bass_utils as bass_utils
from concourse.masks import make_identity
from concourse.library_config import mlp

F32 = mybir.dt.float32
BF16 = mybir.dt.bfloat16
I16 = mybir.dt.int16
I8 = mybir.dt.int8
BF = ml_dtypes.bfloat16

N = 50000
E = 100000
F_IN = 32
F_E = 8
F1 = 64
F2 = 64
KH = 32
FC = 32
NCORES = 8
NPC = N // NCORES
WIN = 128
NWIN = (NPC + WIN - 1) // WIN   # 49
NPC_PAD = NWIN * WIN            # 6272
ST = 8                          # edge tiles per super-tile
TW = 66                         # gathered T row prefix: [xp(64) | a_neigh | 1]

_CACHE = {}


def _wrap_idx(a):
    """int16 index layout for dma_gather: [i%16, i//16], replicated to 128 rows."""
    ni = len(a)
    return np.tile(a.astype(np.int16).reshape(ni // 16, 16).T, (8, 1))


def _host_shard(edge_index):
    """Deal 128-node global windows to cores (balanced by edge-tile count),
    order each core's windows by tile count so per-slot tile counts equalize
    across cores (SPMD needs one structure)."""
    src = np.asarray(edge_index[0], np.int64)
    dst = np.asarray(edge_index[1], np.int64)
    gw = dst // WIN
    ng = (N + WIN - 1) // WIN
    cnt = np.bincount(gw, minlength=ng)
    tiles_g = np.ceil(cnt / 128).astype(np.int64)

    order = np.argsort(-tiles_g, kind="stable")
    core_tiles = np.zeros(NCORES, np.int64)
    core_wins = [[] for _ in range(NCORES)]
    for g in order:
        cands = [c for c in range(NCORES) if len(core_wins[c]) < NWIN]
        c = min(cands, key=lambda c: (core_tiles[c], len(core_wins[c])))
        core_wins[c].append(int(g))
        core_tiles[c] += tiles_g[g]
    gmap = np.full((NCORES, NWIN), -1, np.int64)
    for c in range(NCORES):
        gmap[c, :len(core_wins[c])] = core_wins[c]

    tiles_per_slot = np.zeros(NWIN, np.int64)
    for j in range(NWIN):
        gs = gmap[:, j]
        tiles_per_slot[j] = max(tiles_g[g] if g >= 0 else 0 for g in gs)
    ntiles = int(tiles_per_slot.sum())
    if ntiles % ST:
        tiles_per_slot[0] += ST - ntiles % ST
        ntiles = int(tiles_per_slot.sum())
    e_pad = ntiles * 128
    win_of_tile = np.repeat(np.arange(NWIN), tiles_per_slot)
    slot_base = np.concatenate([[0], np.cumsum(tiles_per_slot * 128)])

    core_of_win = np.full(ng, -1, np.int64)
    slot_of_win = np.full(ng, -1, np.int64)
    for c in range(NCORES):
        for j, g in enumerate(core_wins[c]):
            core_of_win[g] = c
            slot_of_win[g] = j

    e_core = core_of_win[gw]
    e_slot = slot_of_win[gw]
    eid = np.full((NCORES, e_pad), -1, np.int64)
    for c in range(NCORES):
        ids = np.nonzero(e_core == c)[0]
        ids = ids[np.lexsort((dst[ids], e_slot[ids]))]
        sl = e_slot[ids]
        for j in range(NWIN):
            wi = ids[sl == j]
            eid[c, slot_base[j]: slot_base[j] + len(wi)] = wi

    col_of_edge = dst - gw * WIN
    gw_src = src // WIN
    srcT_all = (core_of_win[gw_src] * NPC_PAD + slot_of_win[gw_src] * WIN
                + (src - gw_src * WIN))
    return (eid, win_of_tile, ntiles, e_pad, src, col_of_edge, srcT_all, gmap)


def _host_inputs(inputs):
    x = np.asarray(inputs["x"], np.float32)
    e = np.asarray(inputs["e"], np.float32)
    (eid, win_of_tile, ntiles, e_pad, src, col_of_edge, srcT_all,
     gmap) = _host_shard(inputs["edge_index"])
    nsup = ntiles // ST

    w0 = np.asarray(inputs["ecc_w0"], np.float32)
    b0 = np.asarray(inputs["ecc_b0"], np.float32)
    w1 = np.asarray(inputs["ecc_w1"], np.float32)
    b1 = np.asarray(inputs["ecc_b1"], np.float32)
    root = np.asarray(inputs["ecc_root"], np.float32)
    ecc_bias = np.asarray(inputs["ecc_bias"], np.float32)
    gk = np.asarray(inputs["gat_kernel"], np.float32)
    a_s = np.asarray(inputs["gat_attn_self"], np.float32)
    a_n = np.asarray(inputs["gat_attn_neigh"], np.float32)
    gat_bias = np.asarray(inputs["gat_bias"], np.float32)
    fc_w = np.asarray(inputs["fc_w"], np.float32)
    fc_b = np.asarray(inputs["fc_b"], np.float32)

    use_b0 = bool(np.any(b0))
    use_b1 = bool(np.any(b1))
    ke = F_E + 1 if use_b0 else F_E
    nchunk = 9 if use_b1 else 8

    w0m = np.vstack([w0, b0[None, :]]) if use_b0 else w0
    W1r = w1.reshape(KH, F_IN, F1).reshape(KH * F_IN, F1)
    if use_b1:
        W1r = np.vstack([W1r, b1.reshape(F_IN, F1),
                         np.zeros((128 - F_IN, F1), np.float32)])
    W1re = np.concatenate([W1r[128 * b: 128 * (b + 1)] for b in range(nchunk)],
                          axis=1)
    root_ext = np.vstack([root, ecc_bias[None, :]])
    attn2 = np.stack([a_s, a_n], axis=1)
    gat_bias_rep = np.tile(gat_bias[None, :], (128, 1))
    shared = {
        "w0m": np.ascontiguousarray(w0m.astype(BF)),
        "W1re": np.ascontiguousarray(W1re.astype(BF)),
        "root_ext": np.ascontiguousarray(root_ext),
        "gk": np.ascontiguousarray(gk),
        "attn2": np.ascontiguousarray(attn2),
        "gat_bias_rep": gat_bias_rep,
        "fc_w": np.ascontiguousarray(fc_w),
        "fc_b": np.ascontiguousarray(fc_b.reshape(FC, 1)),
    }

    # per-core compacted x gather table (indices stay < 32768 for int16)
    x128 = np.zeros((N, 128), BF)
    x128[:, :F_IN] = x.astype(BF)

    per_core = []
    uniq_list = []
    for c in range(NCORES):
        ids = eid[c]
        valid = ids >= 0
        idsv = np.where(valid, ids, 0)
        s_glob = np.where(valid, src[idsv], 0)
        uniq, inv = np.unique(s_glob, return_inverse=True)
        uniq_list.append(uniq)
        per_core.append((ids, valid, idsv, s_glob, inv))
    u_pad = int(np.ceil(max(len(u) for u in uniq_list) / 128) * 128)

    in_maps = []
    for c in range(NCORES):
        ids, valid, idsv, s_glob, inv = per_core[c]
        uniq = uniq_list[c]

        e_T = np.where(valid[None, :], e[idsv].T, 0.0)
        if use_b0:
            e_T = np.vstack([e_T, valid[None, :].astype(np.float32)])

        sw = ST * 128
        xg_idx = np.concatenate(
            [_wrap_idx(inv[s * sw:(s + 1) * sw]) for s in range(nsup)], axis=1)
        x_c = np.zeros((u_pad, 128), BF)
        x_c[:len(uniq)] = x128[uniq]

        srcT = np.where(valid, srcT_all[idsv], 0)
        t2_idx = np.concatenate(
            [_wrap_idx(srcT[s * sw:(s + 1) * sw] // 2) for s in range(nsup)], axis=1)
        parity = (srcT % 2).astype(np.int8).reshape(ntiles, 128).T    # [128, ntiles]

        col = np.where(valid, col_of_edge[idsv], -1)
        ok = valid & (col >= 0) & (col < 128)
        S = np.zeros((e_pad, 128), BF)
        S[np.nonzero(ok)[0], col[ok]] = 1.0
        ST_d = np.ascontiguousarray(
            S.reshape(ntiles, 128, 128).transpose(0, 2, 1).reshape(e_pad, 128))

        x_T = np.zeros((F_IN + 1, NPC_PAD), np.float32)
        pool_mask = np.zeros((128, NWIN), np.float32)
        for j in range(NWIN):
            g = gmap[c, j]
            if g < 0:
                continue
            lo = g * WIN
            hi = min(lo + WIN, N)
            x_T[:F_IN, j * WIN: j * WIN + hi - lo] = x[lo:hi].T
            pool_mask[:hi - lo, j] = 1.0
        x_T[F_IN, :] = 1.0

        m = {
            "e_T": np.ascontiguousarray(e_T.astype(BF)),
            "x_c": x_c,
            "xg_idx": np.ascontiguousarray(xg_idx),
            "t2_idx": np.ascontiguousarray(t2_idx),
            "parity": np.ascontiguousarray(parity),
            "S_d": np.ascontiguousarray(S),
            "ST_d": ST_d,
            "x_T": x_T,
            "pool_mask": pool_mask,
        }
        m.update(shared)
        in_maps.append(m)

    meta = dict(ke=ke, nchunk=nchunk, ntiles=ntiles, e_pad=e_pad, u_pad=u_pad,
                win_of_tile=win_of_tile)
    return in_maps, meta


def build_nc(meta, use_collectives=True, num_devices=NCORES):
    ke, nchunk = meta["ke"], meta["nchunk"]
    ntiles, e_pad, u_pad = meta["ntiles"], meta["e_pad"], meta["u_pad"]
    win_of_tile = meta["win_of_tile"]
    nsup = ntiles // ST

    nc = bacc.Bacc("TRN2", target_bir_lowering=False, debug=False,
                   enable_asserts=False, num_devices=num_devices)

    def din(name, shape, dt=F32):
        return nc.dram_tensor(name, shape, dt, kind="ExternalInput").ap()

    e_T = din("e_T", [ke, e_pad], BF16)
    x_c = din("x_c", [u_pad, 128], BF16)
    xg_idx = din("xg_idx", [128, e_pad // 16], I16)
    t2_idx = din("t2_idx", [128, e_pad // 16], I16)
    parity = din("parity", [128, ntiles], I8)
    S_d = din("S_d", [e_pad, 128], BF16)
    ST_dd = din("ST_d", [e_pad, 128], BF16)
    x_T = din("x_T", [F_IN + 1, NPC_PAD])
    w0m = din("w0m", [ke, KH], BF16)
    W1re = din("W1re", [128, F1 * nchunk], BF16)
    root_ext = din("root_ext", [F_IN + 1, F1])
    gk = din("gk", [F2, F2])
    attn2 = din("attn2", [F2, 2])
    gat_bias_rep = din("gat_bias_rep", [128, F2])
    pool_mask = din("pool_mask", [128, NWIN])
    fc_w = din("fc_w", [F2, FC])
    fc_b = din("fc_b", [FC, 1])
    out_d = nc.dram_tensor("out", [FC, 1], F32, kind="ExternalOutput").ap()

    tiles_of_win = [[] for _ in range(NWIN)]
    for t, w in enumerate(win_of_tile):
        tiles_of_win[int(w)].append(t)

    with tile.TileContext(nc) as tc:
        nc.gpsimd.load_library(mlp)
        with (
            tc.tile_pool(name="res", bufs=1) as res,
            tc.tile_pool(name="dram", bufs=1, space="DRAM") as drp,
        ):
            s_all = res.tile([128, ntiles * 128], BF16)
            nc.sync.dma_start(
                s_all[:].rearrange("p (t n) -> p t n", n=128),
                S_d[:].rearrange("(t p) n -> p t n", p=128))
            st_all = res.tile([128, ntiles * 128], BF16)
            nc.sync.dma_start(
                st_all[:].rearrange("p (t n) -> p t n", n=128),
                ST_dd[:].rearrange("(t p) n -> p t n", p=128))
            xgi_sb = res.tile([128, e_pad // 16], I16)
            nc.sync.dma_start(xgi_sb[:], xg_idx[:])
            t2i_sb = res.tile([128, e_pad // 16], I16)
            nc.sync.dma_start(t2i_sb[:], t2_idx[:])
            par_sb = res.tile([128, ntiles], I8)
            nc.sync.dma_start(par_sb[:], parity[:])
            xT_sb = res.tile([F_IN + 1, NPC_PAD], F32)
            nc.sync.dma_start(xT_sb[:], x_T[:])
            w0_sb = res.tile([ke, KH], BF16)
            nc.sync.dma_start(w0_sb[:], w0m[:])
            W1_sb = res.tile([128, F1 * nchunk], BF16)
            nc.sync.dma_start(W1_sb[:], W1re[:])
            root_sb = res.tile([F_IN + 1, F1], F32)
            nc.sync.dma_start(root_sb[:], root_ext[:])
            gk_sb = res.tile([F2, F2], F32)
            nc.sync.dma_start(gk_sb[:], gk[:])
            attn_sb = res.tile([F2, 2], F32)
            nc.sync.dma_start(attn_sb[:], attn2[:])
            gbias_sb = res.tile([128, F2], F32)
            nc.sync.dma_start(gbias_sb[:], gat_bias_rep[:])
            pmask_sb = res.tile([128, NWIN], F32)
            nc.sync.dma_start(pmask_sb[:], pool_mask[:])
            fcw_sb = res.tile([F2, FC], F32)
            nc.sync.dma_start(fcw_sb[:], fc_w[:])
            fcb_sb = res.tile([FC, 1], F32)
            nc.sync.dma_start(fcb_sb[:], fc_b[:])
            ident = res.tile([128, 128], F32)
            make_identity(nc, ident[:])
            ident_bf = res.tile([128, 128], BF16)
            make_identity(nc, ident_bf[:])
            x1_all = res.tile([128, NWIN * F1], F32)
            aself_sb = res.tile([128, NWIN], BF16)
            scores_all = res.tile([128, ntiles], F32)
            ex_all = res.tile([128, ntiles], F32)
            TgM_all = res.tile([128, ntiles * TW], BF16)

            T_loc = drp.tile([NPC_PAD, 128], BF16)
            T_full = drp.tile([NCORES * NPC_PAD, 128], BF16)
            pool_in = drp.tile([F2, 1], F32)
            pool_out = drp.tile([F2, 1], F32)

            def emit_a2(w, sbp, psp):
                x1t_ps = psp.tile([F1, 128], F32, space="PSUM", tag="a2",
                                  name=f"x1t_{w}")
                nc.tensor.transpose(out=x1t_ps[:], in_=x1_all[:, w * F1:(w + 1) * F1],
                                    identity=ident[:])
                x1t_sb = sbp.tile([F1, 128], F32, tag="x1t_sb")
                nc.vector.tensor_copy(x1t_sb[:], x1t_ps[:])
                xpt_ps = psp.tile([F2, 128], F32, space="PSUM", tag="a2",
                                  name=f"xpt_{w}")
                nc.tensor.matmul(out=xpt_ps[:], lhsT=gk_sb[:], rhs=x1t_sb[:],
                                 start=True, stop=True)
                xpt_sb = sbp.tile([F2, 128], F32, tag="xpt_sb")
                nc.scalar.activation(xpt_sb[:], xpt_ps[:],
                                     mybir.ActivationFunctionType.Copy)
                a_ps = psp.tile([2, 128], F32, space="PSUM", tag="a2",
                                name=f"a_{w}")
                nc.tensor.matmul(out=a_ps[:], lhsT=attn_sb[:], rhs=xpt_sb[:],
                                 start=True, stop=True)
                a_sb = sbp.tile([2, 128], F32, tag="a_sb")
                nc.vector.tensor_copy(a_sb[:], a_ps[:])
                xp_ps = psp.tile([128, F2], F32, space="PSUM", tag="a2",
                                 name=f"xp_{w}")
                nc.tensor.transpose(out=xp_ps[:], in_=xpt_sb[:], identity=ident[:F2, :F2])
                Tt = sbp.tile([128, 128], BF16, tag="Tt")
                nc.vector.tensor_copy(Tt[:, :F2], xp_ps[:])
                acol_ps = psp.tile([128, 2], F32, space="PSUM", tag="a2",
                                   name=f"acol_{w}")
                nc.tensor.transpose(out=acol_ps[:], in_=a_sb[:], identity=ident[:2, :2])
                nc.vector.tensor_copy(Tt[:, F2:F2 + 1], acol_ps[:, 1:2])
                nc.vector.memset(Tt[:, F2 + 1:F2 + 2], 1.0)
                nc.vector.memset(Tt[:, F2 + 2:], 0.0)
                nc.vector.tensor_copy(aself_sb[:, w:w + 1], acol_ps[:, 0:1])
                nc.sync.dma_start(T_loc[w * WIN:(w + 1) * WIN, :], Tt[:])

            # ============ Phase A: ECC edges -> x1 ============
            with (
                tc.tile_pool(name="pa_sb", bufs=3) as sa,
                tc.tile_pool(name="pa_big", bufs=2) as sbig,
                tc.tile_pool(name="pa_zt_sb", bufs=1) as szt,
                tc.tile_pool(name="pa_h", bufs=1, space="PSUM") as ph,
                tc.tile_pool(name="pa_zt", bufs=2, space="PSUM") as pzt,
                tc.tile_pool(name="pa_a2", bufs=2, space="PSUM") as pa2,
                tc.tile_pool(name="pa_ms", bufs=1, space="PSUM") as pms,
                tc.tile_pool(name="pa_ag", bufs=2, space="PSUM") as pag,
            ):
                agg_ps = {}
                for s in range(nsup):
                    t0 = s * ST
                    eT_t = sa.tile([ke, ST * 128], BF16, tag="eT")
                    nc.sync.dma_start(eT_t[:], e_T[:, t0 * 128:(t0 + ST) * 128])
                    xg = sa.tile([128, ST * 128], BF16, tag="xg")
                    nc.gpsimd.dma_gather(
                        out_ap=xg[:].rearrange("p (c e) -> p c e", e=128),
                        in_ap=x_c[:], idxs_ap=xgi_sb[:, s * (ST * 8):(s + 1) * (ST * 8)],
                        num_idxs=ST * 128, num_idxs_reg=ST * 128, elem_size=128)
                    h_ps = ph.tile([128, ST * KH], F32, space="PSUM", tag="h")
                    for j in range(ST):
                        nc.tensor.matmul(out=h_ps[:, j * KH:(j + 1) * KH],
                                         lhsT=eT_t[:, j * 128:(j + 1) * 128],
                                         rhs=w0_sb[:], start=True, stop=True)
                    h_sb = sa.tile([128, ST * KH], BF16, tag="h_sb")
                    nc.scalar.activation(h_sb[:], h_ps[:], mybir.ActivationFunctionType.Relu)
                    z = sbig.tile([128, ST * KH * F_IN], BF16, tag="z")
                    for j in range(ST):
                        zv = z[:, j * 1024:(j + 1) * 1024].rearrange("p (k i) -> p k i", k=KH)
                        eng = nc.gpsimd if j in (3, 7) else nc.vector
                        eng.tensor_tensor(
                            out=zv,
                            in0=h_sb[:, j * KH:(j + 1) * KH].unsqueeze(2).broadcast_to([128, KH, F_IN]),
                            in1=xg[:, j * 128:j * 128 + F_IN].unsqueeze(1).broadcast_to([128, KH, F_IN]),
                            op=mybir.AluOpType.mult)
                    zT = szt.tile([128, ST * KH * F_IN], BF16, tag="zT")
                    for j in range(ST):
                        for half in range(2):
                            zt_ps = pzt.tile([128, 512], BF16, space="PSUM", tag="zt")
                            for b in range(4):
                                bb = half * 4 + b
                                nc.tensor.transpose(
                                    out=zt_ps[:, b * 128:(b + 1) * 128],
                                    in_=z[:, j * 1024 + bb * 128: j * 1024 + (bb + 1) * 128],
                                    identity=ident_bf[:])
                            dst_ap = zT[:, j * 1024 + half * 512: j * 1024 + (half + 1) * 512]
                            if (j * 2 + half) % 8 < 6:
                                nc.scalar.activation(dst_ap, zt_ps[:],
                                                     mybir.ActivationFunctionType.Copy)
                            else:
                                nc.vector.tensor_copy(dst_ap, zt_ps[:])
                    if nchunk == 9:
                        z9 = sa.tile([128, ST * 128], BF16, tag="z9")
                        nc.vector.memset(z9[:], 0.0)
                        for j in range(ST):
                            zt_ps9 = pzt.tile([128, 512], BF16, space="PSUM", tag="zt")
                            nc.tensor.transpose(
                                out=zt_ps9[:F_IN, :128],
                                in_=xg[:, j * 128:j * 128 + F_IN],
                                identity=ident_bf[:])
                            nc.vector.tensor_copy(z9[:F_IN, j * 128:(j + 1) * 128],
                                                  zt_ps9[:F_IN, :128])
                    msgs_ps = pms.tile([128, ST * F1], F32, space="PSUM", tag="msgs")
                    for j in range(ST):
                        for b in range(8):
                            nc.tensor.matmul(
                                out=msgs_ps[:, j * F1:(j + 1) * F1],
                                lhsT=zT[:, j * 1024 + b * 128: j * 1024 + (b + 1) * 128],
                                rhs=W1_sb[:, b * F1:(b + 1) * F1],
                                start=(b == 0), stop=(b == nchunk - 1))
                        if nchunk == 9:
                            nc.tensor.matmul(
                                out=msgs_ps[:, j * F1:(j + 1) * F1],
                                lhsT=z9[:, j * 128:(j + 1) * 128],
                                rhs=W1_sb[:, 8 * F1: 9 * F1], start=False, stop=True)
                    msgs_all = sa.tile([128, ST * F1], BF16, tag="msgs_sb")
                    nc.scalar.activation(msgs_all[:], msgs_ps[:],
                                         mybir.ActivationFunctionType.Copy)
                    for j in range(ST):
                        t = t0 + j
                        msgs_sb = msgs_all[:, j * F1:(j + 1) * F1]
                        w = int(win_of_tile[t])
                        if w not in agg_ps:
                            agg_ps[w] = pag.tile([128, F1], F32, space="PSUM",
                                                 tag="agg", name=f"agg_{w}")
                            nc.tensor.matmul(out=agg_ps[w][:],
                                             lhsT=xT_sb[:, w * WIN:(w + 1) * WIN],
                                             rhs=root_sb[:], start=True, stop=False,
                                             skip_group_check=True)
                        nc.tensor.matmul(out=agg_ps[w][:],
                                         lhsT=s_all[:, t * 128:(t + 1) * 128],
                                         rhs=msgs_sb, start=False,
                                         stop=(t == tiles_of_win[w][-1]),
                                         skip_group_check=True)
                        if t == tiles_of_win[w][-1]:
                            nc.scalar.activation(x1_all[:, w * F1:(w + 1) * F1],
                                                 agg_ps[w][:],
                                                 mybir.ActivationFunctionType.Relu)
                            del agg_ps[w]
                            emit_a2(w, sa, pa2)
                for w in range(NWIN):
                    if not tiles_of_win[w]:
                        ap = pag.tile([128, F1], F32, space="PSUM", tag="agg",
                                      name=f"agg_e{w}")
                        nc.tensor.matmul(out=ap[:], lhsT=xT_sb[:, w * WIN:(w + 1) * WIN],
                                         rhs=root_sb[:], start=True, stop=True)
                        nc.scalar.activation(x1_all[:, w * F1:(w + 1) * F1], ap[:],
                                             mybir.ActivationFunctionType.Relu)
                        emit_a2(w, sa, pa2)

            # ============ AllGather T ============
            if use_collectives:
                nc.gpsimd.collective_compute(
                    "AllGather", mybir.AluOpType.bypass,
                    replica_groups=[list(range(NCORES))],
                    ins=[T_loc.opt()], outs=[T_full.opt()])
            else:
                nc.sync.dma_start(T_full[:NPC_PAD, :], T_loc[:])

            T2 = T_full[:].rearrange("(v two) f -> v (two f)", two=2)

            # ============ Phase C: GAT ============
            with (
                tc.tile_pool(name="c_sb", bufs=3) as sc,
                tc.tile_pool(name="c_as", bufs=2, space="PSUM") as pas,
                tc.tile_pool(name="c_o2", bufs=2, space="PSUM") as po2,
                tc.tile_pool(name="c_pool", bufs=1, space="PSUM") as ppl,
            ):
                # C1: gather + merge + scores
                for s in range(nsup):
                    t0 = s * ST
                    Tg = sc.tile([128, ST * 256], BF16, tag="Tg")
                    nc.gpsimd.dma_gather(
                        out_ap=Tg[:].rearrange("p (c e) -> p c e", e=256),
                        in_ap=T2, idxs_ap=t2i_sb[:, s * (ST * 8):(s + 1) * (ST * 8)],
                        num_idxs=ST * 128, num_idxs_reg=ST * 128, elem_size=256)
                    Tg3 = Tg[:].rearrange("p (c e) -> p c e", e=256)
                    selv = TgM_all[:, t0 * TW:(t0 + ST) * TW].rearrange(
                        "p (t f) -> p t f", f=TW)
                    nc.vector.tensor_copy(selv, Tg3[:, :, :TW])
                    nc.vector.copy_predicated(
                        selv,
                        par_sb[:, t0:t0 + ST].unsqueeze(2).broadcast_to([128, ST, TW]),
                        Tg3[:, :, 128:128 + TW])
                    asd_ps = pas.tile([128, ST], F32, space="PSUM", tag="asd")
                    for j in range(ST):
                        t = t0 + j
                        w = int(win_of_tile[t])
                        nc.tensor.matmul(out=asd_ps[:, j:j + 1],
                                         lhsT=st_all[:, t * 128:(t + 1) * 128],
                                         rhs=aself_sb[:, w:w + 1],
                                         start=True, stop=True)
                    nc.vector.tensor_tensor(
                        out=scores_all[:, t0:t0 + ST].unsqueeze(2),
                        in0=asd_ps[:].unsqueeze(2),
                        in1=TgM_all[:, t0 * TW:(t0 + ST) * TW].rearrange(
                            "p (t f) -> p t f", f=TW)[:, :, F2:F2 + 1],
                        op=mybir.AluOpType.add)
                # leaky relu + exp, batched
                lr = sc.tile([128, ntiles], F32, tag="lr")
                nc.vector.tensor_scalar(out=lr[:], in0=scores_all[:], scalar1=0.2,
                                        scalar2=None, op0=mybir.AluOpType.mult)
                nc.vector.tensor_tensor(out=lr[:], in0=lr[:], in1=scores_all[:],
                                        op=mybir.AluOpType.max)
                nc.scalar.activation(ex_all[:], lr[:], mybir.ActivationFunctionType.Exp)

                # C2: weight + scatter + windows
                pool_ps = ppl.tile([F2, 1], F32, space="PSUM", tag="pool")
                out2_ps = {}
                for t in range(ntiles):
                    wm = sc.tile([128, TW], BF16, tag="wm")
                    if t % 2 == 0:
                        nc.vector.tensor_scalar(out=wm[:], in0=TgM_all[:, t * TW:(t + 1) * TW],
                                                scalar1=ex_all[:, t:t + 1], scalar2=None,
                                                op0=mybir.AluOpType.mult)
                    else:
                        nc.scalar.activation(wm[:], TgM_all[:, t * TW:(t + 1) * TW],
                                             mybir.ActivationFunctionType.Copy,
                                             scale=ex_all[:, t:t + 1])
                    w = int(win_of_tile[t])
                    if w not in out2_ps:
                        out2_ps[w] = po2.tile([128, TW], F32, space="PSUM", tag="o2",
                                              name=f"o2_{w}")
                    nc.tensor.matmul(out=out2_ps[w][:],
                                     lhsT=s_all[:, t * 128:(t + 1) * 128],
                                     rhs=wm[:],
                                     start=(t == tiles_of_win[w][0]),
                                     stop=(t == tiles_of_win[w][-1]))
                    if t == tiles_of_win[w][-1]:
                        o2 = out2_ps.pop(w)
                        dn = sc.tile([128, 1], F32, tag="dn")
                        nc.vector.tensor_scalar(out=dn[:], in0=o2[:, TW - 1:TW],
                                                scalar1=1e-9, scalar2=None,
                                                op0=mybir.AluOpType.add)
                        rcp = sc.tile([128, 1], F32, tag="rcp")
                        nc.vector.reciprocal(rcp[:], dn[:])
                        x2 = sc.tile([128, F2], F32, tag="x2")
                        nc.vector.tensor_scalar(out=x2[:], in0=o2[:, :F2],
                                                scalar1=rcp[:, :1], scalar2=None,
                                                op0=mybir.AluOpType.mult)
                        nc.vector.tensor_tensor(out=x2[:], in0=x2[:], in1=gbias_sb[:],
                                                op=mybir.AluOpType.add)
                        nc.scalar.activation(x2[:], x2[:],
                                             mybir.ActivationFunctionType.Relu)
                        nc.tensor.matmul(out=pool_ps[:], lhsT=x2[:],
                                         rhs=pmask_sb[:, w:w + 1],
                                         start=(w == 0), stop=(w == NWIN - 1))

                # ============ Phase D ============
                pooled = sc.tile([F2, 1], F32, tag="pooled")
                nc.scalar.activation(pooled[:], pool_ps[:],
                                     mybir.ActivationFunctionType.Copy, scale=1.0 / N)
                nc.gpsimd.dma_start(pool_in[:], pooled[:])
                if use_collectives:
                    nc.gpsimd.collective_compute(
                        "AllReduce", mybir.AluOpType.add,
                        replica_groups=[list(range(NCORES))],
                        ins=[pool_in.opt()], outs=[pool_out.opt()])
                else:
                    nc.sync.dma_start(pool_out[:], pool_in[:])
                pooled2 = sc.tile([F2, 1], F32, tag="pooled2")
                nc.sync.dma_start(pooled2[:], pool_out[:])
                fc_ps = ppl.tile([FC, 1], F32, space="PSUM", tag="fc")
                nc.tensor.matmul(out=fc_ps[:], lhsT=fcw_sb[:], rhs=pooled2[:],
                                 start=True, stop=True)
                out_sb = sc.tile([FC, 1], F32, tag="out")
                nc.scalar.activation(out_sb[:], fc_ps[:],
                                     mybir.ActivationFunctionType.Relu, bias=fcb_sb[:, :1])
                nc.sync.dma_start(out_d[:], out_sb[:])

    nc.compile()
    return nc


def kernel(**inputs):
    in_maps, meta = _host_inputs(inputs)
    key = (meta["ke"], meta["nchunk"], meta["ntiles"], meta["u_pad"])
    if key not in _CACHE:
        _CACHE[key] = build_nc(meta)
    nc = _CACHE[key]
    res = bass_utils.run_bass_kernel_spmd(nc, in_maps, core_ids=list(range(NCORES)))
    return res.results[0]["out"].reshape(FC).astype(np.float32)
